# revision 1
# baseline (speedup 1.0000x reference)
"""Trainium2 Bass kernel for fused MHA block (nn_MultiHeadAttention_7636451852747).

Reference math (B=2, S=4096, D=512, H=8, hd=64):
    q = (x @ Wq + bq).reshape(B, H, S, hd)   # torch-style .view, no transpose!
    ... scores = q @ k^T / 8; attn = softmax(scores) @ v -> reshape(B,S,D)
    y = LayerNorm(x + attn) * gamma + beta

Key structural insight: the .view(B,H,S,hd) reshape (without transpose) means
head h of batch b only reads rows [h*512, (h+1)*512) of x[b].  The problem
therefore decomposes into B*H = 16 fully independent [512,512] chunks; each of
the 8 cores processes 2 chunks end-to-end with zero inter-core communication.

Within a chunk (x_c = x[b, h*512:(h+1)*512, :], shape [512, 512]):
    q = x_c Wq + bq viewed as Q[4096, 64] with Q[8s+j, d] = q[s, 64j+d]
    scores^T tiles: S_T[(jk,r)][p, s_q] for nk = 8*(128r+p)+jk, nq = 8*s_q+jq
      = matmul(lhsT=k^T[64jk:+64, 128r:+128], rhs=q^T[64jq:+64, :])
    E = exp(S_T/8) in bf16; attn^T strip = sum over (jk,r) of
      matmul(lhsT=[V_tile | ones], rhs=E) -> [65, 512] psum; row 64 = softmax
      denominator (ones-column trick).  PE-transpose [65,128] blocks back to
      natural layout, divide by denominator, add residual, LayerNorm on DVE
      (Newton rsqrt to keep ACT free for exp, which is the bottleneck engine).
All matmuls are bf16 with fp32 PSUM accumulation.
"""
import os
import numpy as np
import ml_dtypes
from contextlib import ExitStack

BF16 = None  # set in _imports
_STATE = {}


def _imports():
    global bass, bacc, tile, mybir, bass_utils, F32, BF16, I32, ALU, ACTF
    import concourse.bass as bass
    import concourse.bacc as bacc
    import concourse.tile as tile
    from concourse import mybir
    from concourse import bass_utils
    F32 = mybir.dt.float32
    BF16 = mybir.dt.bfloat16
    I32 = mybir.dt.int32
    ALU = mybir.AluOpType
    ACTF = mybir.ActivationFunctionType


N_CORES = 8
CHUNKS_PER_CORE = 2
S = 512          # rows per chunk
D = 512          # model dim
HD = 64          # head dim of the viewed [4096, 64] matrices
NQ = 4096        # sub-rows per chunk (S*D/HD)
EPS = 1e-5


def _emit(nc, tc, ctx):
    F32l, BF16l, I32l = F32, BF16, I32
    x_d = nc.dram_tensor("xc", [CHUNKS_PER_CORE, S, D], F32l, kind="ExternalInput").ap()
    xb_d = nc.dram_tensor("xcb", [CHUNKS_PER_CORE, S, D], BF16l, kind="ExternalInput").ap()
    w_d = {n: nc.dram_tensor(n, [D, D], BF16l, kind="ExternalInput").ap()
           for n in ("wq", "wk", "wv")}
    b_d = {n: nc.dram_tensor(n, [1, D], BF16l, kind="ExternalInput").ap()
           for n in ("bq", "bk", "bv")}
    ones_d = nc.dram_tensor("ones", [1, D], BF16l, kind="ExternalInput").ap()
    idf_d = nc.dram_tensor("idf", [128, 128], F32l, kind="ExternalInput").ap()
    gb_d = nc.dram_tensor("gb", [128, D], F32l, kind="ExternalInput").ap()
    bb_d = nc.dram_tensor("bb", [128, D], F32l, kind="ExternalInput").ap()
    y_d = nc.dram_tensor("y", [CHUNKS_PER_CORE, S, D], F32l, kind="ExternalOutput").ap()

    # pools
    consts = ctx.enter_context(tc.tile_pool(name="consts", bufs=1))
    chunkp = ctx.enter_context(tc.tile_pool(name="chunk", bufs=2))
    epool = ctx.enter_context(tc.tile_pool(name="epool", bufs=8))
    attp = ctx.enter_context(tc.tile_pool(name="attp", bufs=2))
    ypool = ctx.enter_context(tc.tile_pool(name="ypool", bufs=3))
    small = ctx.enter_context(tc.tile_pool(name="small", bufs=4))
    # PSUM budget (8 banks): score 2x[128,1024]=4, attn 2, proj 2 (shared
    # with the finalize transposes via the same tag)
    ps_proj = ctx.enter_context(tc.tile_pool(name="ps_proj", bufs=2, space="PSUM"))
    ps_score = ctx.enter_context(tc.tile_pool(name="ps_score", bufs=2, space="PSUM"))
    ps_attn = ctx.enter_context(tc.tile_pool(name="ps_attn", bufs=2, space="PSUM"))

    # ---- constant tiles (DMAs emitted by _consts_early/_late below so the
    # x-transpose DMAs can go FIRST in the single HWDGE queue: the first
    # projection matmul is gated on x^T, not on the weights)
    w_sb = {n: consts.tile([128, 4 * D], BF16l, tag=n, name=f"w_{n}")
            for n in ("wq", "wk", "wv")}
    b_sb = {n: consts.tile([1, D], BF16l, tag=n, name=f"b_{n}")
            for n in ("bq", "bk", "bv")}
    ones = consts.tile([1, D], BF16l, tag="ones")
    idf = consts.tile([128, 128], F32l, tag="idf")
    gb = consts.tile([128, D], F32l, tag="gb")
    bb = consts.tile([128, D], F32l, tag="bb")

    def consts_early():
        for n in ("wq", "wk"):
            for mt in range(4):
                nc.sync.dma_start(w_sb[n][:, 512 * mt:512 * (mt + 1)],
                                  w_d[n][128 * mt:128 * (mt + 1), :])
        for n in ("bq", "bk"):
            nc.sync.dma_start(b_sb[n][:], b_d[n][:])
        nc.sync.dma_start(ones[:], ones_d[:])

    def consts_late():
        for mt in range(4):
            nc.sync.dma_start(w_sb["wv"][:, 512 * mt:512 * (mt + 1)],
                              w_d["wv"][128 * mt:128 * (mt + 1), :])
        nc.sync.dma_start(b_sb["bv"][:], b_d["bv"][:])
        nc.sync.dma_start(idf[:], idf_d[:])
        nc.sync.dma_start(gb[:], gb_d[:])
        nc.sync.dma_start(bb[:], bb_d[:])

    st = [{} for _ in range(CHUNKS_PER_CORE)]  # per-chunk tile state

    def prep_load(c):
        """DMA x; x^T in one hardware DMA transpose (XBAR, bf16).
        dma_start_transpose into a [p, mt, s] view lands source row m at
        partition m%128 of slab m//128 -- exactly the m-tile-major layout."""
        s = st[c]
        s["xT"] = xT = chunkp.tile([128, 4 * D], BF16l, tag="xT", name=f"xT{c}")
        for mt in range(4):
            nc.sync.dma_start_transpose(
                xT[:, 512 * mt:512 * (mt + 1)], xb_d[c][:, 128 * mt:128 * (mt + 1)])
        s["xf"] = xf = chunkp.tile([128, 4 * D], F32l, tag="xf", name=f"xf{c}")
        for t in range(4):
            nc.sync.dma_start(xf[:, 512 * t:512 * (t + 1)], x_d[c, 128 * t:128 * (t + 1), :])
        s["qT"] = chunkp.tile([128, 4 * D], BF16l, tag="qT", name=f"qT{c}")
        s["qTs"] = chunkp.tile([128, 4 * D], BF16l, tag="qTs", name=f"qTs{c}")
        s["kT"] = chunkp.tile([128, 4 * D], BF16l, tag="kT", name=f"kT{c}")
        s["vp"] = chunkp.tile([128, 4 * 520], BF16l, tag="vp", name=f"vp{c}")
        s["h"] = chunkp.tile([128, 4 * D], F32l, tag="h", name=f"h{c}")

    def prep_qk(c, t, which):
        """One q^T or k^T projection column tile (plus qTs swap for q)."""
        s = st[c]
        xT, qT, qTs, kT = s["xT"], s["qT"], s["qTs"], s["kT"]
        wname, bname, dst = (("wq", "bq", qT) if which == "q" else ("wk", "bk", kT))
        pp = ps_proj.tile([128, D], F32l, tag="proj", name=f"pp{c}_{wname}{t}")
        for mt in range(4):
            nc.tensor.matmul(
                pp[:],
                w_sb[wname][:, 512 * mt + 128 * t:512 * mt + 128 * t + 128],
                xT[:, 512 * mt:512 * (mt + 1)],
                start=(mt == 0), stop=False)
        nc.tensor.matmul(pp[:], b_sb[bname][0:1, 128 * t:128 * (t + 1)],
                         ones[0:1, :], start=False, stop=True)
        nc.vector.tensor_copy(dst[0:64, 512 * t:512 * (t + 1)], pp[0:64, :])
        nc.vector.tensor_copy(dst[64:128, 512 * t:512 * (t + 1)], pp[64:128, :])
        if which == "q":
            nc.sync.dma_start(qTs[64:128, 512 * t:512 * (t + 1)], qT[0:64, 512 * t:512 * (t + 1)])
            nc.sync.dma_start(qTs[0:64, 512 * t:512 * (t + 1)], qT[64:128, 512 * t:512 * (t + 1)])

    def prep_v(c, t):
        s = st[c]
        xT, vp = s["xT"], s["vp"]
        pp = ps_proj.tile([128, D], F32l, tag="proj", name=f"pp{c}_v{t}")
        for mt in range(4):
            nc.tensor.matmul(pp[:], xT[:, 512 * mt + 128 * t:512 * mt + 128 * t + 128],
                             w_sb["wv"][:, 512 * mt:512 * (mt + 1)],
                             start=(mt == 0), stop=False)
        nc.tensor.matmul(pp[:], ones[0:1, 0:128], b_sb["bv"][0:1, :],
                         start=False, stop=True)
        blk = vp[:, 520 * t:520 * (t + 1)].rearrange("p (j c) -> p j c", c=65)
        nc.vector.tensor_copy(blk[:, :, 0:64], pp[:].rearrange("p (j c) -> p j c", c=64))
        nc.vector.memset(blk[:, :, 64], 1.0)

    def prep_qkv(c, t):
        prep_qk(c, t, "q")
        prep_qk(c, t, "k")
        prep_v(c, t)

    def strips(c, jp):
        """One jq-pair: scores (row-packed), 1024-wide exp, attn accumulate,
        transpose back + residual."""
        s = st[c]
        qT, qTs, kT, vp, xf, h = s["qT"], s["qTs"], s["kT"], s["vp"], s["xf"], s["h"]

        def qrhs(jq, par):
            src = qT if (jq % 2) == par else qTs
            return src[64 * par:64 * par + 64, 512 * (jq // 2):512 * (jq // 2) + 512]

        jq0, jq1 = 2 * jp, 2 * jp + 1
        pa = [ps_attn.tile([65, D], F32l, tag="attn", name=f"pa{c}_{jp}_{i}")
              for i in range(2)]
        for r in range(4):
            for jku in range(4):
                jk0, jk1 = 2 * jku, 2 * jku + 1
                koff = 512 * jku + 128 * r
                ps0 = ps_score.tile([128, 2 * D], F32l, tag="sps", name=f"s0_{c}_{jp}_{r}_{jku}")
                ps1 = ps_score.tile([128, 2 * D], F32l, tag="sps", name=f"s1_{c}_{jp}_{r}_{jku}")
                nc.tensor.matmul(ps0[:, 0:512], kT[0:64, koff:koff + 128],
                                 qrhs(jq0, 0), start=True, stop=True,
                                 tile_position=(0, 0))
                nc.tensor.matmul(ps1[:, 0:512], kT[64:128, koff:koff + 128],
                                 qrhs(jq0, 1), start=True, stop=True,
                                 tile_position=(64, 0))
                nc.tensor.matmul(ps0[:, 512:1024], kT[0:64, koff:koff + 128],
                                 qrhs(jq1, 0), start=True, stop=True,
                                 tile_position=(0, 0))
                nc.tensor.matmul(ps1[:, 512:1024], kT[64:128, koff:koff + 128],
                                 qrhs(jq1, 1), start=True, stop=True,
                                 tile_position=(64, 0))
                et0 = epool.tile([128, 2 * D], BF16l, tag="e", name=f"e0_{c}_{jp}_{r}_{jku}")
                et1 = epool.tile([128, 2 * D], BF16l, tag="e", name=f"e1_{c}_{jp}_{r}_{jku}")
                nc.scalar.activation(et0[:], ps0[:], ACTF.Exp, scale=0.125)
                nc.scalar.activation(et1[:], ps1[:], ACTF.Exp, scale=0.125)
                first = (r == 0 and jku == 0)
                last = (r == 3 and jku == 3)
                v0 = vp[:, 520 * r + 65 * jk0:520 * r + 65 * jk0 + 65]
                v1 = vp[:, 520 * r + 65 * jk1:520 * r + 65 * jk1 + 65]
                nc.tensor.matmul(pa[0][:], v0, et0[:, 0:512], start=first,
                                 stop=False, skip_group_check=True)
                nc.tensor.matmul(pa[0][:], v1, et1[:, 0:512], start=False,
                                 stop=last, skip_group_check=True)
                nc.tensor.matmul(pa[1][:], v0, et0[:, 512:1024], start=first,
                                 stop=False, skip_group_check=True)
                nc.tensor.matmul(pa[1][:], v1, et1[:, 512:1024], start=False,
                                 stop=last, skip_group_check=True)
        for half, jq in ((0, jq0), (1, jq1)):
            asb = attp.tile([65, D], F32l, tag="asb", name=f"asb{c}_{jp}_{half}")
            nc.vector.tensor_copy(asb[:], pa[half][:])
            for b in range(4):
                tps = ps_proj.tile([128, 65], F32l, tag="proj", name=f"atr{c}_{jp}_{half}_{b}")
                nc.tensor.transpose(tps[:], asb[0:65, 128 * b:128 * (b + 1)],
                                    idf[0:65, 0:65])
                rcp = small.tile([128, 1], F32l, tag="rcp", name=f"rcp{c}_{jp}_{half}_{b}")
                nc.vector.reciprocal(rcp[:], tps[:, 64:65])
                nc.vector.scalar_tensor_tensor(
                    h[:, 512 * b + 64 * jq:512 * b + 64 * jq + 64],
                    tps[:, 0:64], rcp[:],
                    xf[:, 512 * b + 64 * jq:512 * b + 64 * jq + 64],
                    op0=ALU.mult, op1=ALU.add)

    def layer_norm(c):
        """LayerNorm on DVE only; Newton rsqrt batched across the 4 s-tiles."""
        s = st[c]
        h = s["h"]
        mvall = small.tile([128, 8], F32l, tag="mvall", name=f"mv{c}")
        for b in range(4):
            st6 = small.tile([128, 6], F32l, tag="st6", name=f"st6_{c}_{b}")
            nc.vector.bn_stats(st6[:], h[:, 512 * b:512 * (b + 1)])
            nc.vector.bn_aggr(mvall[:, 2 * b:2 * b + 2], st6[:])
        mean4 = mvall[:].rearrange("p (b two) -> p b two", two=2)[:, :, 0]
        var4 = mvall[:].rearrange("p (b two) -> p b two", two=2)[:, :, 1]
        t4 = small.tile([128, 4], F32l, tag="t4", name=f"t4_{c}")
        nc.vector.tensor_scalar_add(t4[:], var4, EPS)
        yi = small.tile([128, 4], I32l, tag="yi", name=f"yi{c}")
        nc.vector.tensor_scalar(yi[:], t4[:].bitcast(I32l), 1, None,
                                op0=ALU.arith_shift_right)
        nc.vector.tensor_scalar(yi[:], yi[:], 0x5F3759DF, -1,
                                op0=ALU.subtract, op1=ALU.mult)
        rstd = small.tile([128, 4], F32l, tag="rstd", name=f"rstd{c}")
        nc.vector.tensor_copy(rstd[:], yi[:].bitcast(F32l))
        y2 = small.tile([128, 4], F32l, tag="y2", name=f"y2_{c}")
        dd = small.tile([128, 4], F32l, tag="dd", name=f"dd{c}")
        for _ in range(3):
            nc.vector.tensor_tensor(y2[:], rstd[:], rstd[:], op=ALU.mult)
            nc.vector.tensor_tensor(y2[:], y2[:], t4[:], op=ALU.mult)
            nc.vector.tensor_scalar(dd[:], y2[:], -0.5, 1.5,
                                    op0=ALU.mult, op1=ALU.add)
            nc.vector.tensor_tensor(rstd[:], rstd[:], dd[:], op=ALU.mult)
        bco = small.tile([128, 4], F32l, tag="bco", name=f"bco{c}")
        nc.vector.tensor_tensor(bco[:], mean4, rstd[:], op=ALU.mult)
        nc.vector.tensor_scalar_mul(bco[:], bco[:], -1.0)
        for b in range(4):
            yt = ypool.tile([128, D], F32l, tag="yt", name=f"yt{c}_{b}")
            nc.vector.tensor_scalar(yt[:], h[:, 512 * b:512 * (b + 1)],
                                    rstd[:, b:b + 1], bco[:, b:b + 1],
                                    op0=ALU.mult, op1=ALU.add)
            nc.vector.tensor_tensor(yt[:], yt[:], gb[:], op=ALU.mult)
            nc.vector.tensor_tensor(yt[:], yt[:], bb[:], op=ALU.add)
            nc.sync.dma_start(y_d[c, 128 * b:128 * (b + 1), :], yt[:])

    # ---- emission schedule: stagger chunk-1 prep into chunk-0's strips so
    # the PE fills ACT-idle gaps with the next chunk's projections.
    prep_load(0)
    consts_early()
    consts_late()
    for t in range(4):
        prep_qkv(0, t)
    strips(0, 0)
    prep_load(1)
    strips(0, 1)
    prep_qk(1, 0, "q")
    prep_qk(1, 0, "k")
    prep_v(1, 0)
    prep_qk(1, 1, "q")
    strips(0, 2)
    prep_qk(1, 1, "k")
    prep_v(1, 1)
    prep_qk(1, 2, "q")
    prep_qk(1, 2, "k")
    strips(0, 3)
    prep_v(1, 2)
    prep_qk(1, 3, "q")
    prep_qk(1, 3, "k")
    prep_v(1, 3)
    layer_norm(0)
    for jp in range(4):
        strips(1, jp)
    layer_norm(1)


def build():
    """Build + compile the Bass module (cached)."""
    if "nc" in _STATE:
        return _STATE["nc"]
    _imports()
    nc = bacc.Bacc("TRN2", target_bir_lowering=False, debug=False,
                   num_devices=N_CORES)
    with tile.TileContext(nc) as tc:
        with ExitStack() as ctx:
            _emit(nc, tc, ctx)
    nc.compile()
    _STATE["nc"] = nc
    return nc


def host_inputs(Wq, bq, Wk, bk, Wv, bv, gamma, beta):
    """Shared per-core constant inputs (everything except x chunks)."""
    bf = ml_dtypes.bfloat16
    base = {
        "wq": np.asarray(Wq, np.float32).astype(bf),
        "wk": np.asarray(Wk, np.float32).astype(bf),
        "wv": np.asarray(Wv, np.float32).astype(bf),
        "bq": np.asarray(bq, np.float32).reshape(1, D).astype(bf),
        "bk": np.asarray(bk, np.float32).reshape(1, D).astype(bf),
        "bv": np.asarray(bv, np.float32).reshape(1, D).astype(bf),
        "ones": np.ones((1, D), bf),
        "idf": np.eye(128, dtype=np.float32),
        "gb": np.broadcast_to(np.asarray(gamma, np.float32), (128, D)).copy(),
        "bb": np.broadcast_to(np.asarray(beta, np.float32), (128, D)).copy(),
    }
    return base


def kernel(x, Wq, bq, Wk, bk, Wv, bv, gamma, beta):
    _imports()
    nc = build()
    x = np.asarray(x, np.float32)
    B, Sfull, Dm = x.shape
    chunks = x.reshape(B * 8, S, D)  # chunk c = (b = c//8, head = c%8)
    bf = ml_dtypes.bfloat16
    base = host_inputs(Wq=Wq, bq=bq, Wk=Wk, bk=bk, Wv=Wv, bv=bv,
                       gamma=gamma, beta=beta)
    in_maps = []
    for i in range(N_CORES):
        xc = np.ascontiguousarray(chunks[2 * i:2 * i + 2])
        m = dict(base)
        m["xc"] = xc
        m["xcb"] = xc.astype(bf)
        in_maps.append(m)
    res = bass_utils.run_bass_kernel_spmd(nc, in_maps, core_ids=list(range(N_CORES)))
    out_chunks = np.empty((B * 8, S, D), np.float32)
    for i in range(N_CORES):
        out_chunks[2 * i:2 * i + 2] = res.results[i]["y"]
    return out_chunks.reshape(B, Sfull, Dm)



# revision 5
# speedup vs baseline: 1.3075x; 1.3075x over previous
"""Trainium2 Bass kernel for fused MHA block (nn_MultiHeadAttention_7636451852747).

Reference math (B=2, S=4096, D=512, H=8, hd=64):
    q = (x @ Wq + bq).view(B, H, 4096, 64)   # torch-style view, no transpose
    scores = q @ k^T / 8; attn = softmax(scores) @ v -> reshape(B, S, D)
    y = LayerNorm(x + attn) * gamma + beta

Structure: the .view means head h of batch b reads only rows [512h, 512h+512)
of x[b]; the problem splits into 16 independent [512,512] chunks, 2 per core.

This version is built around the TRN2 cost model's two dominant terms:
  * PE matmul cost = out_free_rows * cycles_per_row; fp8 DoubleRow runs at
    0.5 cycles/row and contracts 2 k-tiles per pass.  All matmuls (proj,
    scores, attn) are fp8e4m3 DoubleRow.  The attention matmul is flipped
    (E^T as the stationary operand) so the output lands in natural [s, dv]
    layout: no PE transposes, no psum->sbuf attn copy, denominator rides as
    a 65th rhs column of ones.
  * Softmax exp of 16.8M scores/chunk is the bottleneck: split across ACT
    (true exp -> fp8e5m2, bias = ln(scale)) and DVE (Schraudolph bit-trick:
    round(score*A + B) as int8 IS the e5m2 weight, scale-matched to ACT).
    Both read the scores psum directly; a greedy cost balancer assigns
    tiles so both engines stay saturated.
GPSIMD cannot touch PSUM on TRN2, so it stays idle; LayerNorm runs on DVE
(bn_stats + Newton rsqrt) as in the baseline.
"""
import numpy as np
import ml_dtypes
from contextlib import ExitStack

_STATE = {}


def _imports():
    global bass, bacc, tile, mybir, bass_utils, F32, BF16, I8, E4, E5, ALU, ACTF, DR
    import concourse.bass as bass
    import concourse.bacc as bacc
    import concourse.tile as tile
    from concourse import mybir
    from concourse import bass_utils
    F32 = mybir.dt.float32
    BF16 = mybir.dt.bfloat16
    I8 = mybir.dt.int8
    E4 = mybir.dt.float8e4
    E5 = mybir.dt.float8e5
    ALU = mybir.AluOpType
    ACTF = mybir.ActivationFunctionType
    DR = mybir.MatmulPerfMode.DoubleRow


N_CORES = 8
CH = 2           # chunks per core
S = 512          # rows per chunk
D = 512          # model dim
EPS = 1e-5

# Schraudolph-e5m2 constants (calibrated offline vs true softmax):
#   i8 = round(score * EXP_A + EXP_B); bits are the e5m2 weight
#   ACT path: exp(score/8 + LN_S) in e5m2 matches the Schraudolph scale.
EXP_A = 4 * np.log2(np.e) / 8        # 0.7213475
EXP_B = 58.0
LN_S = -0.3095

# emit-time engine cost estimates (us) for the greedy ACT/DVE balancer
C_ACT_EXP = 1.098
C_DVE_EXP = 1.262
C_ACT_CONV = 0.672
C_DVE_CONV = 0.730
C_DVE_VCONV = 0.80
C_DVE_FIN = 0.40
C_DVE_LN = 5.2


def _emit(nc, tc, ctx):
    xt8_d = nc.dram_tensor("xt8", [CH, 128, 2048], E4, kind="ExternalInput").ap()
    xf_d = nc.dram_tensor("xf", [CH, 128, 2048], F32, kind="ExternalInput").ap()
    w8_d = {n: nc.dram_tensor(n, [128, 2048], E4, kind="ExternalInput").ap()
            for n in ("w8q", "w8k", "w8v")}
    bqt_d = nc.dram_tensor("bqt", [128, 4], F32, kind="ExternalInput").ap()
    bkt_d = nc.dram_tensor("bkt", [128, 4], F32, kind="ExternalInput").ap()
    bvb_d = nc.dram_tensor("bvb", [128, D], F32, kind="ExternalInput").ap()
    gb_d = nc.dram_tensor("gb", [128, D], F32, kind="ExternalInput").ap()
    bb_d = nc.dram_tensor("bb", [128, D], F32, kind="ExternalInput").ap()
    y_d = nc.dram_tensor("y", [CH, S, D], F32, kind="ExternalOutput").ap()

    consts = ctx.enter_context(tc.tile_pool(name="consts", bufs=1))
    chunkp = ctx.enter_context(tc.tile_pool(name="chunk", bufs=1))
    epool = ctx.enter_context(tc.tile_pool(name="epool", bufs=2))
    ypool = ctx.enter_context(tc.tile_pool(name="ypool", bufs=3))
    small = ctx.enter_context(tc.tile_pool(name="small", bufs=4))
    ps_proj = ctx.enter_context(tc.tile_pool(name="ps_proj", bufs=2, space="PSUM"))
    ps_score = ctx.enter_context(tc.tile_pool(name="ps_score", bufs=2, space="PSUM"))
    ps_attn = ctx.enter_context(tc.tile_pool(name="ps_attn", bufs=2, space="PSUM"))

    w8 = {n: consts.tile([128, 2048], E4, tag=n, name=f"w_{n}")
          for n in ("w8q", "w8k", "w8v")}
    bqt = consts.tile([128, 4], F32, tag="bqt")
    bkt = consts.tile([128, 4], F32, tag="bkt")
    bvb = consts.tile([128, D], F32, tag="bvb")
    gb = consts.tile([128, D], F32, tag="gb")
    bb = consts.tile([128, D], F32, tag="bb")
    lns = consts.tile([128, 1], F32, tag="lns")

    def consts_dma():
        for n in ("w8q", "w8k"):
            nc.sync.dma_start(w8[n][:], w8_d[n][:])
        nc.sync.dma_start(bqt[:], bqt_d[:])
        nc.sync.dma_start(bkt[:], bkt_d[:])
        nc.vector.memset(lns[:], LN_S)

    def consts_dma_late():
        nc.sync.dma_start(w8["w8v"][:], w8_d["w8v"][:])
        nc.sync.dma_start(bvb[:], bvb_d[:])
        nc.sync.dma_start(gb[:], gb_d[:])
        nc.sync.dma_start(bb[:], bb_d[:])

    # greedy engine balancer (estimated cumulative us per engine)
    bal = {"act": 0.0, "dve": 0.0}

    def pick_engine():
        return "act" if bal["act"] <= bal["dve"] else "dve"

    st = [{} for _ in range(CH)]

    def loads(c):
        s = st[c]
        s["xt8"] = chunkp.tile([128, 2048], E4, tag=f"xt8_{c}", name=f"xt8{c}")
        s["xf"] = chunkp.tile([128, 2048], F32, tag=f"xf_{c}", name=f"xf{c}")
        nc.sync.dma_start(s["xt8"][:], xt8_d[c])
        nc.sync.dma_start(s["xf"][:], xf_d[c])
        s["qT8"] = chunkp.tile([128, 2048], E4, tag=f"qT8_{c}", name=f"qT8{c}")
        s["kT8"] = chunkp.tile([128, 2048], E4, tag=f"kT8_{c}", name=f"kT8{c}")
        s["vp"] = chunkp.tile([128, 4 * 520], E4, tag=f"vp_{c}", name=f"vp{c}")
        s["qd"] = chunkp.tile([128, 8192], E4, tag=f"qd_{c}", name=f"qd{c}")
        s["kd"] = chunkp.tile([128, 8192], E4, tag=f"kd_{c}", name=f"kd{c}")
        s["h"] = chunkp.tile([128, 2048], F32, tag=f"h_{c}", name=f"h{c}")

    def proj(c):
        s = st[c]
        xt_v = s["xt8"][:].rearrange("p (st i s) -> p st i s", st=2, i=2)
        for t in range(4):
            for which in ("q", "k", "v"):
                pp = ps_proj.tile([128, D], F32, tag="proj",
                                  name=f"pp{c}_{which}{t}")
                if which == "v":
                    w_v = w8["w8v"][:].rearrange("p (st i m) -> p st i m", st=2, i=2)
                    for step in range(2):
                        nc.tensor.matmul(
                            pp[:], xt_v[:, step, :, 128 * t:128 * (t + 1)],
                            w_v[:, step], start=(step == 0), stop=(step == 1),
                            perf_mode=DR)
                    blk = s["vp"][:].rearrange("p (t j c) -> p t j c", j=8, c=65)
                    nc.vector.tensor_tensor(
                        blk[:, t, :, 0:64],
                        pp[:].rearrange("p (j c) -> p j c", c=64),
                        bvb[:].rearrange("p (j c) -> p j c", c=64), op=ALU.add)
                    nc.vector.memset(blk[:, t, :, 64], 1.0)
                    bal["dve"] += C_DVE_VCONV
                else:
                    wname = "w8q" if which == "q" else "w8k"
                    dst = s["qT8"] if which == "q" else s["kT8"]
                    bias = bqt if which == "q" else bkt
                    w_v = w8[wname][:].rearrange("p (st i m) -> p st i m",
                                                 st=2, i=2)
                    for step in range(2):
                        nc.tensor.matmul(
                            pp[:], w_v[:, step, :, 128 * t:128 * (t + 1)],
                            xt_v[:, step], start=(step == 0), stop=(step == 1),
                            perf_mode=DR)
                    eng = pick_engine()
                    if eng == "act":
                        nc.scalar.activation(dst[:, 512 * t:512 * (t + 1)],
                                             pp[:], ACTF.Identity,
                                             bias=bias[:, t:t + 1])
                        bal["act"] += C_ACT_CONV
                    else:
                        nc.vector.tensor_scalar(dst[:, 512 * t:512 * (t + 1)],
                                                pp[:], bias[:, t:t + 1], None,
                                                op0=ALU.add)
                        bal["dve"] += C_DVE_CONV

    def regroup(c):
        s = st[c]
        for mat, src in (("qd", "qT8"), ("kd", "kT8")):
            dst_v = s[mat][0:32, :].rearrange(
                "p (t par i s) -> p par i t s", par=2, i=2, s=512)
            for par in range(2):
                for i in range(2):
                    base = 64 * par + 32 * i
                    nc.sync.dma_start(
                        dst_v[:, par, i],
                        s[src][base:base + 32, :].rearrange("p (t s) -> p t s",
                                                            s=512))

    def emit_exp(dst, ps):
        eng = pick_engine()
        if eng == "act":
            nc.scalar.activation(dst, ps, ACTF.Exp, scale=0.125, bias=lns[:])
            bal["act"] += C_ACT_EXP
        else:
            nc.vector.tensor_scalar(dst.bitcast(I8), ps, EXP_A, EXP_B,
                                    op0=ALU.mult, op1=ALU.add)
            bal["dve"] += C_DVE_EXP

    def strips(c, jp):
        """Scores + exp for jq pair (2jp, 2jp+1): 16 (r,jku) psum pairs."""
        s = st[c]
        qd_v = s["qd"][0:32, :].rearrange("p (jq i s) -> p jq i s", i=2, s=512)
        kd_v = s["kd"][0:32, :].rearrange("p (jk i s) -> p jk i s", i=2, s=512)
        ets = []
        for r in range(4):
            for jku in range(4):
                ps0 = ps_score.tile([128, 1024], F32, tag="sps",
                                    name=f"s0_{c}_{jp}_{r}_{jku}")
                ps1 = ps_score.tile([128, 1024], F32, tag="sps",
                                    name=f"s1_{c}_{jp}_{r}_{jku}")
                for pjq in range(2):
                    rhsq = qd_v[:, 2 * jp + pjq]
                    nc.tensor.matmul(
                        ps0[:, 512 * pjq:512 * (pjq + 1)],
                        kd_v[:, 2 * jku, :, 128 * r:128 * (r + 1)], rhsq,
                        start=True, stop=True, perf_mode=DR)
                for pjq in range(2):
                    rhsq = qd_v[:, 2 * jp + pjq]
                    nc.tensor.matmul(
                        ps1[:, 512 * pjq:512 * (pjq + 1)],
                        kd_v[:, 2 * jku + 1, :, 128 * r:128 * (r + 1)], rhsq,
                        start=True, stop=True, perf_mode=DR)
                et = epool.tile([128, 2048], E5, tag=f"e{4 * r + jku}",
                                name=f"e_{c}_{jp}_{r}_{jku}")
                emit_exp(et[:, 0:1024], ps0[:])
                emit_exp(et[:, 1024:2048], ps1[:])
                ets.append(et)
        return ets

    def attn_fin(c, jp, ets):
        """Flipped attention (E^T stationary) + finalize into h."""
        s = st[c]
        vp_v = s["vp"][:].rearrange("p (t j c) -> p t j c", j=8, c=65)
        for sb in range(4):
            for pjq in range(2):
                jq = 2 * jp + pjq
                pa = ps_attn.tile([128, 512], F32, tag="pa",
                                  name=f"pa_{c}_{jp}_{sb}_{pjq}")
                k = 0
                for r in range(4):
                    for jku in range(4):
                        et = ets[4 * r + jku]
                        lhsT = et[:].rearrange("p (i m) -> p i m", i=2)[
                            :, :, 512 * pjq + 128 * sb:512 * pjq + 128 * sb + 128]
                        rhs = vp_v[:, r, 2 * jku:2 * jku + 2, :]
                        nc.tensor.matmul(pa[:, 0:65], lhsT, rhs,
                                         start=(k == 0), stop=(k == 15),
                                         skip_group_check=True, perf_mode=DR)
                        k += 1
                rcp = small.tile([128, 1], F32, tag="rcp",
                                 name=f"rcp_{c}_{jp}_{sb}_{pjq}")
                nc.vector.reciprocal(rcp[:], pa[:, 64:65])
                off = 512 * sb + 64 * jq
                nc.vector.scalar_tensor_tensor(
                    s["h"][:, off:off + 64], pa[:, 0:64], rcp[:],
                    s["xf"][:, off:off + 64], op0=ALU.mult, op1=ALU.add)
                bal["dve"] += C_DVE_FIN

    def layer_norm(c):
        """LayerNorm on DVE; Newton rsqrt batched across the 4 s-blocks."""
        s = st[c]
        h = s["h"]
        I32 = mybir.dt.int32
        mvall = small.tile([128, 8], F32, tag="mvall", name=f"mv{c}")
        for b in range(4):
            st6 = small.tile([128, 6], F32, tag="st6", name=f"st6_{c}_{b}")
            nc.vector.bn_stats(st6[:], h[:, 512 * b:512 * (b + 1)])
            nc.vector.bn_aggr(mvall[:, 2 * b:2 * b + 2], st6[:])
        mean4 = mvall[:].rearrange("p (b two) -> p b two", two=2)[:, :, 0]
        var4 = mvall[:].rearrange("p (b two) -> p b two", two=2)[:, :, 1]
        t4 = small.tile([128, 4], F32, tag="t4", name=f"t4_{c}")
        nc.vector.tensor_scalar_add(t4[:], var4, EPS)
        yi = small.tile([128, 4], I32, tag="yi", name=f"yi{c}")
        nc.vector.tensor_scalar(yi[:], t4[:].bitcast(I32), 1, None,
                                op0=ALU.arith_shift_right)
        nc.vector.tensor_scalar(yi[:], yi[:], 0x5F3759DF, -1,
                                op0=ALU.subtract, op1=ALU.mult)
        rstd = small.tile([128, 4], F32, tag="rstd", name=f"rstd{c}")
        nc.vector.tensor_copy(rstd[:], yi[:].bitcast(F32))
        y2 = small.tile([128, 4], F32, tag="y2", name=f"y2_{c}")
        dd = small.tile([128, 4], F32, tag="dd", name=f"dd{c}")
        for _ in range(3):
            nc.vector.tensor_tensor(y2[:], rstd[:], rstd[:], op=ALU.mult)
            nc.vector.tensor_tensor(y2[:], y2[:], t4[:], op=ALU.mult)
            nc.vector.tensor_scalar(dd[:], y2[:], -0.5, 1.5,
                                    op0=ALU.mult, op1=ALU.add)
            nc.vector.tensor_tensor(rstd[:], rstd[:], dd[:], op=ALU.mult)
        bco = small.tile([128, 4], F32, tag="bco", name=f"bco{c}")
        nc.vector.tensor_tensor(bco[:], mean4, rstd[:], op=ALU.mult)
        nc.vector.tensor_scalar_mul(bco[:], bco[:], -1.0)
        for b in range(4):
            yt = ypool.tile([128, D], F32, tag="yt", name=f"yt{c}_{b}")
            nc.vector.tensor_scalar(yt[:], h[:, 512 * b:512 * (b + 1)],
                                    rstd[:, b:b + 1], bco[:, b:b + 1],
                                    op0=ALU.mult, op1=ALU.add)
            nc.vector.tensor_tensor(yt[:], yt[:], gb[:], op=ALU.mult)
            nc.vector.tensor_tensor(yt[:], yt[:], bb[:], op=ALU.add)
            nc.sync.dma_start(y_d[c, 128 * b:128 * (b + 1), :], yt[:])
        bal["dve"] += C_DVE_LN

    # ---- emission schedule: PE order = proj(0), scores(0,0..3) with
    # attn(jp-1) slotted between strips, proj(1), attn(0,3), scores(1,*) ...
    consts_dma()
    loads(0)
    consts_dma_late()
    proj(0)
    regroup(0)
    loads(1)
    pend = None   # (c, jp, ets) awaiting attn+finalize
    for c in range(CH):
        if c == 1:
            proj(1)
            regroup(1)
            attn_fin(*pend)
            pend = None
            layer_norm(0)
        for jp in range(4):
            ets = strips(c, jp)
            if pend is not None:
                attn_fin(*pend)
            pend = (c, jp, ets)
    attn_fin(*pend)
    layer_norm(1)


def build():
    if "nc" in _STATE:
        return _STATE["nc"]
    _imports()
    nc = bacc.Bacc("TRN2", target_bir_lowering=False, debug=False,
                   num_devices=N_CORES)
    with tile.TileContext(nc) as tc:
        with ExitStack() as ctx:
            _emit(nc, tc, ctx)
    nc.compile()
    _STATE["nc"] = nc
    return nc


def host_inputs(Wq, bq, Wk, bk, Wv, bv, gamma, beta):
    """Shared per-core constant inputs (everything except x chunks)."""
    e4 = ml_dtypes.float8_e4m3

    def pack_w(W):
        W = np.asarray(W, np.float32)
        return np.ascontiguousarray(
            W.reshape(2, 2, 128, 512).transpose(2, 0, 1, 3).reshape(128, 2048)
        ).astype(e4)

    def bias_t(b):
        return np.ascontiguousarray(
            np.asarray(b, np.float32).reshape(4, 128).T)

    return {
        "w8q": pack_w(Wq), "w8k": pack_w(Wk), "w8v": pack_w(Wv),
        "bqt": bias_t(bq), "bkt": bias_t(bk),
        "bvb": np.broadcast_to(np.asarray(bv, np.float32), (128, D)).copy(),
        "gb": np.broadcast_to(np.asarray(gamma, np.float32), (128, D)).copy(),
        "bb": np.broadcast_to(np.asarray(beta, np.float32), (128, D)).copy(),
    }


def kernel(x, Wq, bq, Wk, bk, Wv, bv, gamma, beta):
    _imports()
    nc = build()
    e4 = ml_dtypes.float8_e4m3
    x = np.asarray(x, np.float32)
    B, Sfull, Dm = x.shape
    chunks = x.reshape(B * 8, S, D)  # chunk c = (b = c//8, head = c%8)
    base = host_inputs(Wq=Wq, bq=bq, Wk=Wk, bk=bk, Wv=Wv, bv=bv,
                       gamma=gamma, beta=beta)
    in_maps = []
    for i in range(N_CORES):
        xc = chunks[2 * i:2 * i + 2]                       # [2, 512, 512]
        xt8 = np.ascontiguousarray(
            xc.transpose(0, 2, 1).reshape(CH, 4, 128, S).transpose(0, 2, 1, 3)
            .reshape(CH, 128, 2048)).astype(e4)            # x^T, m-tile-major
        xf = np.ascontiguousarray(
            xc.reshape(CH, 4, 128, D).transpose(0, 2, 1, 3)
            .reshape(CH, 128, 2048))                       # residual, s-block-major
        m = dict(base)
        m["xt8"] = xt8
        m["xf"] = xf
        in_maps.append(m)
    res = bass_utils.run_bass_kernel_spmd(nc, in_maps, core_ids=list(range(N_CORES)))
    out_chunks = np.empty((B * 8, S, D), np.float32)
    for i in range(N_CORES):
        out_chunks[2 * i:2 * i + 2] = res.results[i]["y"]
    return out_chunks.reshape(B, Sfull, Dm)


# revision 6
# speedup vs baseline: 1.3947x; 1.0667x over previous
"""Trainium2 Bass kernel for fused MHA block (nn_MultiHeadAttention_7636451852747).

Reference math (B=2, S=4096, D=512, H=8, hd=64):
    q = (x @ Wq + bq).view(B, H, 4096, 64)   # torch-style view, no transpose
    scores = q @ k^T / 8; attn = softmax(scores) @ v -> reshape(B, S, D)
    y = LayerNorm(x + attn) * gamma + beta

Structure: the .view means head h of batch b reads only rows [512h, 512h+512)
of x[b]; the problem splits into 16 independent [512,512] chunks, 2 per core.

This version is built around the TRN2 cost model's two dominant terms:
  * PE matmul cost = out_free_rows * cycles_per_row; fp8 DoubleRow runs at
    0.5 cycles/row and contracts 2 k-tiles per pass.  All matmuls (proj,
    scores, attn) are fp8e4m3 DoubleRow.  The attention matmul is flipped
    (E^T as the stationary operand) so the output lands in natural [s, dv]
    layout: no PE transposes, no psum->sbuf attn copy, denominator rides as
    a 65th rhs column of ones.
  * Softmax exp of 16.8M scores/chunk is the bottleneck: split across ACT
    (true exp -> fp8e5m2, bias = ln(scale)) and DVE (Schraudolph bit-trick:
    round(score*A + B) as int8 IS the e5m2 weight, scale-matched to ACT).
    Both read the scores psum directly; a greedy cost balancer assigns
    tiles so both engines stay saturated.
GPSIMD cannot touch PSUM on TRN2, so it stays idle; LayerNorm runs on DVE
(bn_stats + Newton rsqrt) as in the baseline.
"""
import numpy as np
import ml_dtypes
from contextlib import ExitStack

_STATE = {}


def _imports():
    global bass, bacc, tile, mybir, bass_utils, F32, BF16, I8, E4, E5, ALU, ACTF, DR
    import concourse.bass as bass
    import concourse.bacc as bacc
    import concourse.tile as tile
    from concourse import mybir
    from concourse import bass_utils
    F32 = mybir.dt.float32
    BF16 = mybir.dt.bfloat16
    I8 = mybir.dt.int8
    E4 = mybir.dt.float8e4
    E5 = mybir.dt.float8e5
    ALU = mybir.AluOpType
    ACTF = mybir.ActivationFunctionType
    DR = mybir.MatmulPerfMode.DoubleRow


N_CORES = 8
CH = 2           # chunks per core
S = 512          # rows per chunk
D = 512          # model dim
EPS = 1e-5

# Schraudolph-e5m2 constants (calibrated offline vs true softmax):
#   i8 = round(score * EXP_A + EXP_B); bits are the e5m2 weight
#   ACT path: exp(score/8 + LN_S) in e5m2 matches the Schraudolph scale.
EXP_A = 4 * np.log2(np.e) / 8        # 0.7213475
EXP_B = 58.0
LN_S = -0.3095

# emit-time engine cost estimates (us) for the greedy ACT/DVE balancer
C_ACT_EXP = 1.098
C_DVE_EXP = 1.262
C_ACT_CONV = 0.672
C_DVE_CONV = 0.730
C_DVE_VCONV = 0.80
C_DVE_FIN = 0.40
C_DVE_LN = 5.2


def _emit(nc, tc, ctx):
    xt8_d = nc.dram_tensor("xt8", [CH, 128, 2048], E4, kind="ExternalInput").ap()
    xf_d = nc.dram_tensor("xf", [CH, 128, 2048], F32, kind="ExternalInput").ap()
    w8_d = {n: nc.dram_tensor(n, [128, 2048], E4, kind="ExternalInput").ap()
            for n in ("w8q", "w8k", "w8v")}
    bqt_d = nc.dram_tensor("bqt", [128, 4], F32, kind="ExternalInput").ap()
    bkt_d = nc.dram_tensor("bkt", [128, 4], F32, kind="ExternalInput").ap()
    bvb_d = nc.dram_tensor("bvb", [128, D], F32, kind="ExternalInput").ap()
    gb_d = nc.dram_tensor("gb", [128, D], F32, kind="ExternalInput").ap()
    bb_d = nc.dram_tensor("bb", [128, D], F32, kind="ExternalInput").ap()
    y_d = nc.dram_tensor("y", [CH, S, D], F32, kind="ExternalOutput").ap()

    consts = ctx.enter_context(tc.tile_pool(name="consts", bufs=1))
    chunkp = ctx.enter_context(tc.tile_pool(name="chunk", bufs=1))
    epool = ctx.enter_context(tc.tile_pool(name="epool", bufs=2))
    ypool = ctx.enter_context(tc.tile_pool(name="ypool", bufs=3))
    small = ctx.enter_context(tc.tile_pool(name="small", bufs=4))
    ps_proj = ctx.enter_context(tc.tile_pool(name="ps_proj", bufs=1, space="PSUM"))
    ps_score = ctx.enter_context(tc.tile_pool(name="ps_score", bufs=3, space="PSUM"))
    ps_attn = ctx.enter_context(tc.tile_pool(name="ps_attn", bufs=1, space="PSUM"))

    w8 = {n: consts.tile([128, 2048], E4, tag=n, name=f"w_{n}")
          for n in ("w8q", "w8k", "w8v")}
    bqt = consts.tile([128, 4], F32, tag="bqt")
    bkt = consts.tile([128, 4], F32, tag="bkt")
    bvb = consts.tile([128, D], F32, tag="bvb")
    gb = consts.tile([128, D], F32, tag="gb")
    bb = consts.tile([128, D], F32, tag="bb")
    lns = consts.tile([128, 1], F32, tag="lns")

    def consts_dma():
        for n in ("w8q", "w8k"):
            nc.sync.dma_start(w8[n][:], w8_d[n][:])
        nc.sync.dma_start(bqt[:], bqt_d[:])
        nc.sync.dma_start(bkt[:], bkt_d[:])
        nc.vector.memset(lns[:], LN_S)

    def consts_dma_late():
        nc.sync.dma_start(w8["w8v"][:], w8_d["w8v"][:])
        nc.sync.dma_start(bvb[:], bvb_d[:])
        nc.sync.dma_start(gb[:], gb_d[:])
        nc.sync.dma_start(bb[:], bb_d[:])

    # greedy engine balancer (estimated cumulative us per engine)
    # pre-charge DVE with its fixed (non-exp) future work so the greedy
    # exp split accounts for finalize/LN/v-convert from the start
    bal = {"act": 0.0,
           "dve": CH * (C_DVE_LN + 32 * C_DVE_FIN + 4 * C_DVE_VCONV)}

    def pick_engine():
        return "act" if bal["act"] <= bal["dve"] else "dve"

    st = [{} for _ in range(CH)]

    def loads(c):
        s = st[c]
        s["xt8"] = chunkp.tile([128, 2048], E4, tag=f"xt8_{c}", name=f"xt8{c}")
        s["xf"] = chunkp.tile([128, 2048], F32, tag=f"xf_{c}", name=f"xf{c}")
        nc.sync.dma_start(s["xt8"][:], xt8_d[c])
        nc.sync.dma_start(s["xf"][:], xf_d[c])
        s["qT8"] = chunkp.tile([128, 2048], E4, tag=f"qT8_{c}", name=f"qT8{c}")
        s["kT8"] = chunkp.tile([128, 2048], E4, tag=f"kT8_{c}", name=f"kT8{c}")
        s["vp"] = chunkp.tile([128, 4 * 520], E4, tag=f"vp_{c}", name=f"vp{c}")
        s["qd"] = chunkp.tile([128, 8192], E4, tag=f"qd_{c}", name=f"qd{c}")
        s["kd"] = chunkp.tile([128, 8192], E4, tag=f"kd_{c}", name=f"kd{c}")
        s["h"] = chunkp.tile([128, 2048], F32, tag=f"h_{c}", name=f"h{c}")

    def proj(c):
        s = st[c]
        xt_v = s["xt8"][:].rearrange("p (st i s) -> p st i s", st=2, i=2)
        for t in range(4):
            for which in ("q", "k", "v"):
                pp = ps_proj.tile([128, D], F32, tag="proj",
                                  name=f"pp{c}_{which}{t}")
                if which == "v":
                    w_v = w8["w8v"][:].rearrange("p (st i m) -> p st i m", st=2, i=2)
                    for step in range(2):
                        nc.tensor.matmul(
                            pp[:], xt_v[:, step, :, 128 * t:128 * (t + 1)],
                            w_v[:, step], start=(step == 0), stop=(step == 1),
                            perf_mode=DR)
                    blk = s["vp"][:].rearrange("p (t j c) -> p t j c", j=8, c=65)
                    nc.vector.tensor_tensor(
                        blk[:, t, :, 0:64],
                        pp[:].rearrange("p (j c) -> p j c", c=64),
                        bvb[:].rearrange("p (j c) -> p j c", c=64), op=ALU.add)
                    nc.vector.memset(blk[:, t, :, 64], 1.0)
                else:
                    wname = "w8q" if which == "q" else "w8k"
                    dst = s["qT8"] if which == "q" else s["kT8"]
                    bias = bqt if which == "q" else bkt
                    w_v = w8[wname][:].rearrange("p (st i m) -> p st i m",
                                                 st=2, i=2)
                    for step in range(2):
                        nc.tensor.matmul(
                            pp[:], w_v[:, step, :, 128 * t:128 * (t + 1)],
                            xt_v[:, step], start=(step == 0), stop=(step == 1),
                            perf_mode=DR)
                    eng = pick_engine()
                    if eng == "act":
                        nc.scalar.activation(dst[:, 512 * t:512 * (t + 1)],
                                             pp[:], ACTF.Identity,
                                             bias=bias[:, t:t + 1])
                        bal["act"] += C_ACT_CONV
                    else:
                        nc.vector.tensor_scalar(dst[:, 512 * t:512 * (t + 1)],
                                                pp[:], bias[:, t:t + 1], None,
                                                op0=ALU.add)
                        bal["dve"] += C_DVE_CONV

    def regroup(c):
        s = st[c]
        for mat, src in (("qd", "qT8"), ("kd", "kT8")):
            dst_v = s[mat][0:32, :].rearrange(
                "p (t par i s) -> p par i t s", par=2, i=2, s=512)
            for par in range(2):
                for i in range(2):
                    base = 64 * par + 32 * i
                    nc.sync.dma_start(
                        dst_v[:, par, i],
                        s[src][base:base + 32, :].rearrange("p (t s) -> p t s",
                                                            s=512))

    def emit_exp(dst, ps):
        eng = pick_engine()
        if eng == "act":
            nc.scalar.activation(dst, ps, ACTF.Exp, scale=0.125, bias=lns[:])
            bal["act"] += C_ACT_EXP
        else:
            nc.vector.tensor_scalar(dst.bitcast(I8), ps, EXP_A, EXP_B,
                                    op0=ALU.mult, op1=ALU.add)
            bal["dve"] += C_DVE_EXP

    def strips(c, jp):
        """Scores + exp for jq pair (2jp, 2jp+1): 16 (r,jku) psum pairs."""
        s = st[c]
        qd_v = s["qd"][0:32, :].rearrange("p (jq i s) -> p jq i s", i=2, s=512)
        kd_v = s["kd"][0:32, :].rearrange("p (jk i s) -> p jk i s", i=2, s=512)
        ets = []
        for r in range(4):
            for jku in range(4):
                ps0 = ps_score.tile([128, 1024], F32, tag="sps",
                                    name=f"s0_{c}_{jp}_{r}_{jku}")
                ps1 = ps_score.tile([128, 1024], F32, tag="sps",
                                    name=f"s1_{c}_{jp}_{r}_{jku}")
                for pjq in range(2):
                    rhsq = qd_v[:, 2 * jp + pjq]
                    nc.tensor.matmul(
                        ps0[:, 512 * pjq:512 * (pjq + 1)],
                        kd_v[:, 2 * jku, :, 128 * r:128 * (r + 1)], rhsq,
                        start=True, stop=True, perf_mode=DR)
                for pjq in range(2):
                    rhsq = qd_v[:, 2 * jp + pjq]
                    nc.tensor.matmul(
                        ps1[:, 512 * pjq:512 * (pjq + 1)],
                        kd_v[:, 2 * jku + 1, :, 128 * r:128 * (r + 1)], rhsq,
                        start=True, stop=True, perf_mode=DR)
                et = epool.tile([128, 2048], E5, tag=f"e{4 * r + jku}",
                                name=f"e_{c}_{jp}_{r}_{jku}")
                emit_exp(et[:, 0:1024], ps0[:])
                emit_exp(et[:, 1024:2048], ps1[:])
                ets.append(et)
        return ets

    def attn_fin(c, jp, ets):
        """Flipped attention (E^T stationary) + finalize into h."""
        s = st[c]
        vp_v = s["vp"][:].rearrange("p (t j c) -> p t j c", j=8, c=65)
        for sb in range(4):
            for pjq in range(2):
                jq = 2 * jp + pjq
                pa = ps_attn.tile([128, 512], F32, tag="pa",
                                  name=f"pa_{c}_{jp}_{sb}_{pjq}")
                k = 0
                for r in range(4):
                    for jku in range(4):
                        et = ets[4 * r + jku]
                        lhsT = et[:].rearrange("p (i m) -> p i m", i=2)[
                            :, :, 512 * pjq + 128 * sb:512 * pjq + 128 * sb + 128]
                        rhs = vp_v[:, r, 2 * jku:2 * jku + 2, :]
                        nc.tensor.matmul(pa[:, 0:65], lhsT, rhs,
                                         start=(k == 0), stop=(k == 15),
                                         skip_group_check=True, perf_mode=DR)
                        k += 1
                rcp = small.tile([128, 1], F32, tag="rcp",
                                 name=f"rcp_{c}_{jp}_{sb}_{pjq}")
                nc.vector.reciprocal(rcp[:], pa[:, 64:65])
                off = 512 * sb + 64 * jq
                nc.vector.scalar_tensor_tensor(
                    s["h"][:, off:off + 64], pa[:, 0:64], rcp[:],
                    s["xf"][:, off:off + 64], op0=ALU.mult, op1=ALU.add)

    def layer_norm(c):
        """LayerNorm on DVE; Newton rsqrt batched across the 4 s-blocks."""
        s = st[c]
        h = s["h"]
        I32 = mybir.dt.int32
        mvall = small.tile([128, 8], F32, tag="mvall", name=f"mv{c}")
        for b in range(4):
            st6 = small.tile([128, 6], F32, tag="st6", name=f"st6_{c}_{b}")
            nc.vector.bn_stats(st6[:], h[:, 512 * b:512 * (b + 1)])
            nc.vector.bn_aggr(mvall[:, 2 * b:2 * b + 2], st6[:])
        mean4 = mvall[:].rearrange("p (b two) -> p b two", two=2)[:, :, 0]
        var4 = mvall[:].rearrange("p (b two) -> p b two", two=2)[:, :, 1]
        t4 = small.tile([128, 4], F32, tag="t4", name=f"t4_{c}")
        nc.vector.tensor_scalar_add(t4[:], var4, EPS)
        yi = small.tile([128, 4], I32, tag="yi", name=f"yi{c}")
        nc.vector.tensor_scalar(yi[:], t4[:].bitcast(I32), 1, None,
                                op0=ALU.arith_shift_right)
        nc.vector.tensor_scalar(yi[:], yi[:], 0x5F3759DF, -1,
                                op0=ALU.subtract, op1=ALU.mult)
        rstd = small.tile([128, 4], F32, tag="rstd", name=f"rstd{c}")
        nc.vector.tensor_copy(rstd[:], yi[:].bitcast(F32))
        y2 = small.tile([128, 4], F32, tag="y2", name=f"y2_{c}")
        dd = small.tile([128, 4], F32, tag="dd", name=f"dd{c}")
        for _ in range(3):
            nc.vector.tensor_tensor(y2[:], rstd[:], rstd[:], op=ALU.mult)
            nc.vector.tensor_tensor(y2[:], y2[:], t4[:], op=ALU.mult)
            nc.vector.tensor_scalar(dd[:], y2[:], -0.5, 1.5,
                                    op0=ALU.mult, op1=ALU.add)
            nc.vector.tensor_tensor(rstd[:], rstd[:], dd[:], op=ALU.mult)
        bco = small.tile([128, 4], F32, tag="bco", name=f"bco{c}")
        nc.vector.tensor_tensor(bco[:], mean4, rstd[:], op=ALU.mult)
        nc.vector.tensor_scalar_mul(bco[:], bco[:], -1.0)
        for b in range(4):
            yt = ypool.tile([128, D], F32, tag="yt", name=f"yt{c}_{b}")
            nc.vector.tensor_scalar(yt[:], h[:, 512 * b:512 * (b + 1)],
                                    rstd[:, b:b + 1], bco[:, b:b + 1],
                                    op0=ALU.mult, op1=ALU.add)
            nc.vector.tensor_tensor(yt[:], yt[:], gb[:], op=ALU.mult)
            nc.vector.tensor_tensor(yt[:], yt[:], bb[:], op=ALU.add)
            nc.sync.dma_start(y_d[c, 128 * b:128 * (b + 1), :], yt[:])

    # ---- emission schedule: PE order = proj(0), scores(0,0..3) with
    # attn(jp-1) slotted between strips, proj(1), attn(0,3), scores(1,*) ...
    consts_dma()
    loads(0)
    consts_dma_late()
    proj(0)
    regroup(0)
    loads(1)
    pend = None   # (c, jp, ets) awaiting attn+finalize
    for c in range(CH):
        if c == 1:
            proj(1)
            regroup(1)
            attn_fin(*pend)
            pend = None
            layer_norm(0)
        for jp in range(4):
            ets = strips(c, jp)
            if pend is not None:
                attn_fin(*pend)
            pend = (c, jp, ets)
    attn_fin(*pend)
    layer_norm(1)


def build():
    if "nc" in _STATE:
        return _STATE["nc"]
    _imports()
    nc = bacc.Bacc("TRN2", target_bir_lowering=False, debug=False,
                   num_devices=N_CORES)
    with tile.TileContext(nc) as tc:
        with ExitStack() as ctx:
            _emit(nc, tc, ctx)
    nc.compile()
    _STATE["nc"] = nc
    return nc


def host_inputs(Wq, bq, Wk, bk, Wv, bv, gamma, beta):
    """Shared per-core constant inputs (everything except x chunks)."""
    e4 = ml_dtypes.float8_e4m3

    def pack_w(W):
        W = np.asarray(W, np.float32)
        return np.ascontiguousarray(
            W.reshape(2, 2, 128, 512).transpose(2, 0, 1, 3).reshape(128, 2048)
        ).astype(e4)

    def bias_t(b):
        return np.ascontiguousarray(
            np.asarray(b, np.float32).reshape(4, 128).T)

    return {
        "w8q": pack_w(Wq), "w8k": pack_w(Wk), "w8v": pack_w(Wv),
        "bqt": bias_t(bq), "bkt": bias_t(bk),
        "bvb": np.broadcast_to(np.asarray(bv, np.float32), (128, D)).copy(),
        "gb": np.broadcast_to(np.asarray(gamma, np.float32), (128, D)).copy(),
        "bb": np.broadcast_to(np.asarray(beta, np.float32), (128, D)).copy(),
    }


def kernel(x, Wq, bq, Wk, bk, Wv, bv, gamma, beta):
    _imports()
    nc = build()
    e4 = ml_dtypes.float8_e4m3
    x = np.asarray(x, np.float32)
    B, Sfull, Dm = x.shape
    chunks = x.reshape(B * 8, S, D)  # chunk c = (b = c//8, head = c%8)
    base = host_inputs(Wq=Wq, bq=bq, Wk=Wk, bk=bk, Wv=Wv, bv=bv,
                       gamma=gamma, beta=beta)
    in_maps = []
    for i in range(N_CORES):
        xc = chunks[2 * i:2 * i + 2]                       # [2, 512, 512]
        xt8 = np.ascontiguousarray(
            xc.transpose(0, 2, 1).reshape(CH, 4, 128, S).transpose(0, 2, 1, 3)
            .reshape(CH, 128, 2048)).astype(e4)            # x^T, m-tile-major
        xf = np.ascontiguousarray(
            xc.reshape(CH, 4, 128, D).transpose(0, 2, 1, 3)
            .reshape(CH, 128, 2048))                       # residual, s-block-major
        m = dict(base)
        m["xt8"] = xt8
        m["xf"] = xf
        in_maps.append(m)
    res = bass_utils.run_bass_kernel_spmd(nc, in_maps, core_ids=list(range(N_CORES)))
    out_chunks = np.empty((B * 8, S, D), np.float32)
    for i in range(N_CORES):
        out_chunks[2 * i:2 * i + 2] = res.results[i]["y"]
    return out_chunks.reshape(B, Sfull, Dm)


# revision 11
# speedup vs baseline: 1.5518x; 1.1126x over previous
"""Trainium2 Bass kernel for fused MHA block (nn_MultiHeadAttention_7636451852747).

Reference math (B=2, S=4096, D=512, H=8, hd=64):
    q = (x @ Wq + bq).view(B, H, 4096, 64)   # torch-style view, no transpose
    scores = q @ k^T / 8; attn = softmax(scores) @ v -> reshape(B, S, D)
    y = LayerNorm(x + attn) * gamma + beta

Structure: the .view means head h of batch b reads only rows [512h, 512h+512)
of x[b]; the problem splits into 16 independent [512,512] chunks, 2 per core.

This version is built around the TRN2 cost model's two dominant terms:
  * PE matmul cost = out_free_rows * cycles_per_row; fp8 DoubleRow runs at
    0.5 cycles/row and contracts 2 k-tiles per pass.  All matmuls (proj,
    scores, attn) are fp8e4m3 DoubleRow.  The attention matmul is flipped
    (E^T as the stationary operand) so the output lands in natural [s, dv]
    layout: no PE transposes, no psum->sbuf attn copy, denominator rides as
    a 65th rhs column of ones.
  * Softmax exp of 16.8M scores/chunk is the bottleneck: split across ACT
    (true exp -> fp8e5m2, bias = ln(scale)) and DVE (Schraudolph bit-trick:
    round(score*A + B) as int8 IS the e5m2 weight, scale-matched to ACT).
    Both read the scores psum directly; a greedy cost balancer assigns
    tiles so both engines stay saturated.
GPSIMD cannot touch PSUM on TRN2, so it stays idle; LayerNorm runs on DVE
(bn_stats + Newton rsqrt) as in the baseline.
"""
import numpy as np
import ml_dtypes
from contextlib import ExitStack

_STATE = {}


def _imports():
    global bass, bacc, tile, mybir, bass_utils, F32, BF16, I8, E4, E5, ALU, ACTF, DR
    import concourse.bass as bass
    import concourse.bacc as bacc
    import concourse.tile as tile
    from concourse import mybir
    from concourse import bass_utils
    F32 = mybir.dt.float32
    BF16 = mybir.dt.bfloat16
    I8 = mybir.dt.int8
    E4 = mybir.dt.float8e4
    E5 = mybir.dt.float8e5
    ALU = mybir.AluOpType
    ACTF = mybir.ActivationFunctionType
    DR = mybir.MatmulPerfMode.DoubleRow


N_CORES = 8
CH = 2           # chunks per core
S = 512          # rows per chunk
D = 512          # model dim
EPS = 1e-5

# Schraudolph-e5m2 constants (calibrated offline vs true softmax):
#   i8 = round(score * EXP_A + EXP_B); bits are the e5m2 weight
#   ACT path: exp(score/8 + LN_S) in e5m2 matches the Schraudolph scale.
EXP_A = 4 * np.log2(np.e) / 8        # 0.7213475
EXP_B = 58.0
LN_S = -0.3095

# emit-time engine cost estimates (us) for the greedy ACT/DVE balancer
C_ACT_EXP = 1.098
C_DVE_EXP = 1.262
C_ACT_CONV = 0.672
C_DVE_CONV = 0.730
C_DVE_VCONV = 0.80
C_DVE_FIN = 0.40
C_DVE_LN = 5.2


def _emit(nc, tc, ctx):
    xt8_d = nc.dram_tensor("xt8", [CH, 128, 2048], E4, kind="ExternalInput").ap()
    xf_d = nc.dram_tensor("xf", [CH, 128, 2048], F32, kind="ExternalInput").ap()
    w8_d = {n: nc.dram_tensor(n, [128, 2048], E4, kind="ExternalInput").ap()
            for n in ("w8q", "w8k", "w8v")}
    bqt_d = nc.dram_tensor("bqt", [128, 4], F32, kind="ExternalInput").ap()
    bkt_d = nc.dram_tensor("bkt", [128, 4], F32, kind="ExternalInput").ap()
    bvb_d = nc.dram_tensor("bvb", [128, D], F32, kind="ExternalInput").ap()
    gb_d = nc.dram_tensor("gb", [128, D], F32, kind="ExternalInput").ap()
    bb_d = nc.dram_tensor("bb", [128, D], F32, kind="ExternalInput").ap()
    y_d = nc.dram_tensor("y", [CH, S, D], F32, kind="ExternalOutput").ap()

    consts = ctx.enter_context(tc.tile_pool(name="consts", bufs=1))
    chunkp = ctx.enter_context(tc.tile_pool(name="chunk", bufs=1))
    epool = ctx.enter_context(tc.tile_pool(name="epool", bufs=2))
    ypool = ctx.enter_context(tc.tile_pool(name="ypool", bufs=3))
    small = ctx.enter_context(tc.tile_pool(name="small", bufs=4))
    ps_proj = ctx.enter_context(tc.tile_pool(name="ps_proj", bufs=1, space="PSUM"))
    ps_score = ctx.enter_context(tc.tile_pool(name="ps_score", bufs=3, space="PSUM"))
    ps_attn = ctx.enter_context(tc.tile_pool(name="ps_attn", bufs=1, space="PSUM"))

    w8 = {n: consts.tile([128, 2048], E4, tag=n, name=f"w_{n}")
          for n in ("w8q", "w8k", "w8v")}
    bqt = consts.tile([128, 4], F32, tag="bqt")
    bkt = consts.tile([128, 4], F32, tag="bkt")
    bvb = consts.tile([128, D], F32, tag="bvb")
    gb = consts.tile([128, D], F32, tag="gb")
    bb = consts.tile([128, D], F32, tag="bb")
    lns = consts.tile([128, 1], F32, tag="lns")

    def consts_dma():
        for n in ("w8q", "w8k"):
            nc.sync.dma_start(w8[n][:], w8_d[n][:])
        nc.sync.dma_start(bqt[:], bqt_d[:])
        nc.sync.dma_start(bkt[:], bkt_d[:])
        nc.vector.memset(lns[:], LN_S)

    def consts_dma_late():
        nc.sync.dma_start(w8["w8v"][:], w8_d["w8v"][:])
        nc.sync.dma_start(bvb[:], bvb_d[:])
        nc.sync.dma_start(gb[:], gb_d[:])
        nc.sync.dma_start(bb[:], bb_d[:])

    # greedy engine balancer (estimated cumulative us per engine)
    bal = {"act": 0.0, "dve": 0.0}

    def pick_engine():
        return "act" if bal["act"] <= bal["dve"] else "dve"

    st = [{} for _ in range(CH)]

    def loads(c):
        s = st[c]
        s["xt8"] = chunkp.tile([128, 2048], E4, tag=f"xt8_{c}", name=f"xt8{c}")
        s["xf"] = chunkp.tile([128, 2048], F32, tag=f"xf_{c}", name=f"xf{c}")
        nc.sync.dma_start(s["xt8"][:], xt8_d[c])
        nc.sync.dma_start(s["xf"][:], xf_d[c])
        s["qT8"] = chunkp.tile([128, 2048], E4, tag=f"qT8_{c}", name=f"qT8{c}")
        s["kT8"] = chunkp.tile([128, 2048], E4, tag=f"kT8_{c}", name=f"kT8{c}")
        s["vp"] = chunkp.tile([128, 4 * 520], E4, tag=f"vp_{c}", name=f"vp{c}")
        s["qd"] = chunkp.tile([128, 8192], E4, tag=f"qd_{c}", name=f"qd{c}")
        s["kd"] = chunkp.tile([128, 8192], E4, tag=f"kd_{c}", name=f"kd{c}")
        s["h"] = chunkp.tile([128, 2048], F32, tag=f"h_{c}", name=f"h{c}")

    def proj(c):
        s = st[c]
        xt_v = s["xt8"][:].rearrange("p (st i s) -> p st i s", st=2, i=2)
        for t in range(4):
            for which in ("q", "k", "v"):
                pp = ps_proj.tile([128, D], F32, tag="proj",
                                  name=f"pp{c}_{which}{t}")
                if which == "v":
                    w_v = w8["w8v"][:].rearrange("p (st i m) -> p st i m", st=2, i=2)
                    for step in range(2):
                        nc.tensor.matmul(
                            pp[:], xt_v[:, step, :, 128 * t:128 * (t + 1)],
                            w_v[:, step], start=(step == 0), stop=(step == 1),
                            perf_mode=DR)
                    blk = s["vp"][:].rearrange("p (t j c) -> p t j c", j=8, c=65)
                    nc.vector.tensor_tensor(
                        blk[:, t, :, 0:64],
                        pp[:].rearrange("p (j c) -> p j c", c=64),
                        bvb[:].rearrange("p (j c) -> p j c", c=64), op=ALU.add)
                    nc.vector.memset(blk[:, t, :, 64], 1.0)
                else:
                    wname = "w8q" if which == "q" else "w8k"
                    dst = s["qT8"] if which == "q" else s["kT8"]
                    bias = bqt if which == "q" else bkt
                    w_v = w8[wname][:].rearrange("p (st i m) -> p st i m",
                                                 st=2, i=2)
                    for step in range(2):
                        nc.tensor.matmul(
                            pp[:], w_v[:, step, :, 128 * t:128 * (t + 1)],
                            xt_v[:, step], start=(step == 0), stop=(step == 1),
                            perf_mode=DR)
                    eng = pick_engine()
                    if eng == "act":
                        nc.scalar.activation(dst[:, 512 * t:512 * (t + 1)],
                                             pp[:], ACTF.Identity,
                                             bias=bias[:, t:t + 1])
                        bal["act"] += C_ACT_CONV
                    else:
                        nc.vector.tensor_scalar(dst[:, 512 * t:512 * (t + 1)],
                                                pp[:], bias[:, t:t + 1], None,
                                                op0=ALU.add)
                        bal["dve"] += C_DVE_CONV

    def regroup(c):
        s = st[c]
        for mat, src in (("qd", "qT8"), ("kd", "kT8")):
            dst_v = s[mat][0:32, :].rearrange(
                "p (t par i s) -> p par i t s", par=2, i=2, s=512)
            for par in range(2):
                for i in range(2):
                    base = 64 * par + 32 * i
                    nc.sync.dma_start(
                        dst_v[:, par, i],
                        s[src][base:base + 32, :].rearrange("p (t s) -> p t s",
                                                            s=512))

    # DVE's fixed non-exp work, spread as a per-exp-tile handicap so the
    # greedy split tilts toward ACT smoothly instead of in one early burst
    fixed_dve = CH * (C_DVE_LN + 32 * C_DVE_FIN + 4 * C_DVE_VCONV)
    handicap = fixed_dve / (CH * 128.0)

    def emit_exp(dst, ps):
        bal["dve"] += handicap
        eng = pick_engine()
        if eng == "act":
            nc.scalar.activation(dst, ps, ACTF.Exp, scale=0.125, bias=lns[:])
            bal["act"] += C_ACT_EXP
        else:
            nc.vector.tensor_scalar(dst.bitcast(I8), ps, EXP_A, EXP_B,
                                    op0=ALU.mult, op1=ALU.add)
            bal["dve"] += C_DVE_EXP

    def strips(c, jp):
        """Scores + exp for jq pair (2jp, 2jp+1): 16 (r,jku) psum pairs."""
        s = st[c]
        qd_v = s["qd"][0:32, :].rearrange("p (jq i s) -> p jq i s", i=2, s=512)
        kd_v = s["kd"][0:32, :].rearrange("p (jk i s) -> p jk i s", i=2, s=512)
        ets = []
        for r in range(4):
            for jku in range(4):
                ps0 = ps_score.tile([128, 1024], F32, tag="sps",
                                    name=f"s0_{c}_{jp}_{r}_{jku}")
                ps1 = ps_score.tile([128, 1024], F32, tag="sps",
                                    name=f"s1_{c}_{jp}_{r}_{jku}")
                for pjq in range(2):
                    rhsq = qd_v[:, 2 * jp + pjq]
                    nc.tensor.matmul(
                        ps0[:, 512 * pjq:512 * (pjq + 1)],
                        kd_v[:, 2 * jku, :, 128 * r:128 * (r + 1)], rhsq,
                        start=True, stop=True, perf_mode=DR)
                for pjq in range(2):
                    rhsq = qd_v[:, 2 * jp + pjq]
                    nc.tensor.matmul(
                        ps1[:, 512 * pjq:512 * (pjq + 1)],
                        kd_v[:, 2 * jku + 1, :, 128 * r:128 * (r + 1)], rhsq,
                        start=True, stop=True, perf_mode=DR)
                et = epool.tile([128, 2048], E5, tag=f"e{4 * r + jku}",
                                name=f"e_{c}_{jp}_{r}_{jku}")
                emit_exp(et[:, 0:1024], ps0[:])
                emit_exp(et[:, 1024:2048], ps1[:])
                ets.append(et)
        return ets

    def attn_fin(c, jp, ets, use_sps=False):
        """Flipped attention (E^T stationary) + finalize into h.

        use_sps: draw the accumulators from the (then idle) score psum pool
        for 3-deep pipelining — only safe when no more scores will run."""
        s = st[c]
        vp_v = s["vp"][:].rearrange("p (t j c) -> p t j c", j=8, c=65)
        for sb in range(4):
            for pjq in range(2):
                jq = 2 * jp + pjq
                if use_sps:
                    pa = ps_score.tile([128, 1024], F32, tag="sps",
                                       name=f"pa_{c}_{jp}_{sb}_{pjq}")
                else:
                    pa = ps_attn.tile([128, 512], F32, tag="pa",
                                      name=f"pa_{c}_{jp}_{sb}_{pjq}")
                k = 0
                for r in range(4):
                    for jku in range(4):
                        et = ets[4 * r + jku]
                        lhsT = et[:].rearrange("p (i m) -> p i m", i=2)[
                            :, :, 512 * pjq + 128 * sb:512 * pjq + 128 * sb + 128]
                        rhs = vp_v[:, r, 2 * jku:2 * jku + 2, :]
                        nc.tensor.matmul(pa[:, 0:65], lhsT, rhs,
                                         start=(k == 0), stop=(k == 15),
                                         skip_group_check=True, perf_mode=DR)
                        k += 1
                rcp = small.tile([128, 1], F32, tag="rcp",
                                 name=f"rcp_{c}_{jp}_{sb}_{pjq}")
                nc.vector.reciprocal(rcp[:], pa[:, 64:65])
                off = 512 * sb + 64 * jq
                nc.vector.scalar_tensor_tensor(
                    s["h"][:, off:off + 64], pa[:, 0:64], rcp[:],
                    s["xf"][:, off:off + 64], op0=ALU.mult, op1=ALU.add)

    def layer_norm(c):
        """LayerNorm on DVE; Newton rsqrt batched across the 4 s-blocks."""
        s = st[c]
        h = s["h"]
        I32 = mybir.dt.int32
        mvall = small.tile([128, 8], F32, tag="mvall", name=f"mv{c}")
        for b in range(4):
            st6 = small.tile([128, 6], F32, tag="st6", name=f"st6_{c}_{b}")
            nc.vector.bn_stats(st6[:], h[:, 512 * b:512 * (b + 1)])
            nc.vector.bn_aggr(mvall[:, 2 * b:2 * b + 2], st6[:])
        mean4 = mvall[:].rearrange("p (b two) -> p b two", two=2)[:, :, 0]
        var4 = mvall[:].rearrange("p (b two) -> p b two", two=2)[:, :, 1]
        t4 = small.tile([128, 4], F32, tag="t4", name=f"t4_{c}")
        nc.vector.tensor_scalar_add(t4[:], var4, EPS)
        yi = small.tile([128, 4], I32, tag="yi", name=f"yi{c}")
        nc.vector.tensor_scalar(yi[:], t4[:].bitcast(I32), 1, None,
                                op0=ALU.arith_shift_right)
        nc.vector.tensor_scalar(yi[:], yi[:], 0x5F3759DF, -1,
                                op0=ALU.subtract, op1=ALU.mult)
        rstd = small.tile([128, 4], F32, tag="rstd", name=f"rstd{c}")
        nc.vector.tensor_copy(rstd[:], yi[:].bitcast(F32))
        y2 = small.tile([128, 4], F32, tag="y2", name=f"y2_{c}")
        dd = small.tile([128, 4], F32, tag="dd", name=f"dd{c}")
        for _ in range(3):
            nc.vector.tensor_tensor(y2[:], rstd[:], rstd[:], op=ALU.mult)
            nc.vector.tensor_tensor(y2[:], y2[:], t4[:], op=ALU.mult)
            nc.vector.tensor_scalar(dd[:], y2[:], -0.5, 1.5,
                                    op0=ALU.mult, op1=ALU.add)
            nc.vector.tensor_tensor(rstd[:], rstd[:], dd[:], op=ALU.mult)
        bco = small.tile([128, 4], F32, tag="bco", name=f"bco{c}")
        nc.vector.tensor_tensor(bco[:], mean4, rstd[:], op=ALU.mult)
        nc.vector.tensor_scalar_mul(bco[:], bco[:], -1.0)
        for b in range(4):
            yt = ypool.tile([128, D], F32, tag="yt", name=f"yt{c}_{b}")
            # stage 1 (h*rstd + bco) on ACT to shorten the DVE-only tail
            nc.scalar.activation(yt[:], h[:, 512 * b:512 * (b + 1)],
                                 ACTF.Identity, bias=bco[:, b:b + 1],
                                 scale=rstd[:, b:b + 1])
            nc.vector.tensor_tensor(yt[:], yt[:], gb[:], op=ALU.mult)
            nc.vector.tensor_tensor(yt[:], yt[:], bb[:], op=ALU.add)
            nc.sync.dma_start(y_d[c, 128 * b:128 * (b + 1), :], yt[:])

    # ---- emission schedule: PE order = proj(0), scores(0,0..3) with
    # attn(jp-1) slotted between strips, proj(1), attn(0,3), scores(1,*) ...
    consts_dma()
    loads(0)
    consts_dma_late()
    proj(0)
    regroup(0)
    loads(1)
    pend = None   # (c, jp, ets) awaiting attn+finalize
    for c in range(CH):
        if c == 1:
            proj(1)
            regroup(1)
            attn_fin(*pend)
            pend = None
            layer_norm(0)
        for jp in range(4):
            ets = strips(c, jp)
            if pend is not None:
                attn_fin(*pend)
            pend = (c, jp, ets)
    attn_fin(*pend, use_sps=True)
    layer_norm(1)


def build():
    if "nc" in _STATE:
        return _STATE["nc"]
    _imports()
    nc = bacc.Bacc("TRN2", target_bir_lowering=False, debug=False,
                   num_devices=N_CORES)
    with tile.TileContext(nc) as tc:
        with ExitStack() as ctx:
            _emit(nc, tc, ctx)
    nc.compile()
    _STATE["nc"] = nc
    return nc


def host_inputs(Wq, bq, Wk, bk, Wv, bv, gamma, beta):
    """Shared per-core constant inputs (everything except x chunks)."""
    e4 = ml_dtypes.float8_e4m3

    def pack_w(W):
        W = np.asarray(W, np.float32)
        return np.ascontiguousarray(
            W.reshape(2, 2, 128, 512).transpose(2, 0, 1, 3).reshape(128, 2048)
        ).astype(e4)

    def bias_t(b):
        return np.ascontiguousarray(
            np.asarray(b, np.float32).reshape(4, 128).T)

    return {
        "w8q": pack_w(Wq), "w8k": pack_w(Wk), "w8v": pack_w(Wv),
        "bqt": bias_t(bq), "bkt": bias_t(bk),
        "bvb": np.broadcast_to(np.asarray(bv, np.float32), (128, D)).copy(),
        "gb": np.broadcast_to(np.asarray(gamma, np.float32), (128, D)).copy(),
        "bb": np.broadcast_to(np.asarray(beta, np.float32), (128, D)).copy(),
    }


def kernel(x, Wq, bq, Wk, bk, Wv, bv, gamma, beta):
    _imports()
    nc = build()
    e4 = ml_dtypes.float8_e4m3
    x = np.asarray(x, np.float32)
    B, Sfull, Dm = x.shape
    chunks = x.reshape(B * 8, S, D)  # chunk c = (b = c//8, head = c%8)
    base = host_inputs(Wq=Wq, bq=bq, Wk=Wk, bk=bk, Wv=Wv, bv=bv,
                       gamma=gamma, beta=beta)
    in_maps = []
    for i in range(N_CORES):
        xc = chunks[2 * i:2 * i + 2]                       # [2, 512, 512]
        xt8 = np.ascontiguousarray(
            xc.transpose(0, 2, 1).reshape(CH, 4, 128, S).transpose(0, 2, 1, 3)
            .reshape(CH, 128, 2048)).astype(e4)            # x^T, m-tile-major
        xf = np.ascontiguousarray(
            xc.reshape(CH, 4, 128, D).transpose(0, 2, 1, 3)
            .reshape(CH, 128, 2048))                       # residual, s-block-major
        m = dict(base)
        m["xt8"] = xt8
        m["xf"] = xf
        in_maps.append(m)
    res = bass_utils.run_bass_kernel_spmd(nc, in_maps, core_ids=list(range(N_CORES)))
    out_chunks = np.empty((B * 8, S, D), np.float32)
    for i in range(N_CORES):
        out_chunks[2 * i:2 * i + 2] = res.results[i]["y"]
    return out_chunks.reshape(B, Sfull, Dm)


# revision 12
# speedup vs baseline: 1.5529x; 1.0008x over previous
"""Trainium2 Bass kernel for fused MHA block (nn_MultiHeadAttention_7636451852747).

Reference math (B=2, S=4096, D=512, H=8, hd=64):
    q = (x @ Wq + bq).view(B, H, 4096, 64)   # torch-style view, no transpose
    scores = q @ k^T / 8; attn = softmax(scores) @ v -> reshape(B, S, D)
    y = LayerNorm(x + attn) * gamma + beta

Structure: the .view means head h of batch b reads only rows [512h, 512h+512)
of x[b]; the problem splits into 16 independent [512,512] chunks, 2 per core.

This version is built around the TRN2 cost model's two dominant terms:
  * PE matmul cost = out_free_rows * cycles_per_row; fp8 DoubleRow runs at
    0.5 cycles/row and contracts 2 k-tiles per pass.  All matmuls (proj,
    scores, attn) are fp8e4m3 DoubleRow.  The attention matmul is flipped
    (E^T as the stationary operand) so the output lands in natural [s, dv]
    layout: no PE transposes, no psum->sbuf attn copy, denominator rides as
    a 65th rhs column of ones.
  * Softmax exp of 16.8M scores/chunk is the bottleneck: split across ACT
    (true exp -> fp8e5m2, bias = ln(scale)) and DVE (Schraudolph bit-trick:
    round(score*A + B) as int8 IS the e5m2 weight, scale-matched to ACT).
    Both read the scores psum directly; a greedy cost balancer assigns
    tiles so both engines stay saturated.
GPSIMD cannot touch PSUM on TRN2, so it stays idle; LayerNorm runs on DVE
(bn_stats + Newton rsqrt) as in the baseline.
"""
import numpy as np
import ml_dtypes
from contextlib import ExitStack

_STATE = {}


def _imports():
    global bass, bacc, tile, mybir, bass_utils, F32, BF16, I8, E4, E5, ALU, ACTF, DR
    import concourse.bass as bass
    import concourse.bacc as bacc
    import concourse.tile as tile
    from concourse import mybir
    from concourse import bass_utils
    F32 = mybir.dt.float32
    BF16 = mybir.dt.bfloat16
    I8 = mybir.dt.int8
    E4 = mybir.dt.float8e4
    E5 = mybir.dt.float8e5
    ALU = mybir.AluOpType
    ACTF = mybir.ActivationFunctionType
    DR = mybir.MatmulPerfMode.DoubleRow


N_CORES = 8
CH = 2           # chunks per core
S = 512          # rows per chunk
D = 512          # model dim
EPS = 1e-5

# Schraudolph-e5m2 constants (calibrated offline vs true softmax):
#   i8 = round(score * EXP_A + EXP_B); bits are the e5m2 weight
#   ACT path: exp(score/8 + LN_S) in e5m2 matches the Schraudolph scale.
EXP_A = 4 * np.log2(np.e) / 8        # 0.7213475
EXP_B = 58.0
LN_S = -0.3095

# emit-time engine cost estimates (us) for the greedy ACT/DVE balancer
C_ACT_EXP = 1.098
C_DVE_EXP = 1.262
C_ACT_CONV = 0.672
C_DVE_CONV = 0.730
C_DVE_VCONV = 0.80
C_DVE_FIN = 0.40
C_DVE_LN = 5.2


def _emit(nc, tc, ctx):
    xtb_d = nc.dram_tensor("xtb", [CH, 128, 2048], BF16, kind="ExternalInput").ap()
    xf_d = nc.dram_tensor("xf", [CH, 128, 2048], F32, kind="ExternalInput").ap()
    wb_d = {n: nc.dram_tensor(n, [128, 2048], BF16, kind="ExternalInput").ap()
            for n in ("wbq", "wbk", "wbv")}
    bqt_d = nc.dram_tensor("bqt", [128, 4], F32, kind="ExternalInput").ap()
    bkt_d = nc.dram_tensor("bkt", [128, 4], F32, kind="ExternalInput").ap()
    bvb_d = nc.dram_tensor("bvb", [128, D], F32, kind="ExternalInput").ap()
    gb_d = nc.dram_tensor("gb", [128, D], F32, kind="ExternalInput").ap()
    bb_d = nc.dram_tensor("bb", [128, D], F32, kind="ExternalInput").ap()
    y_d = nc.dram_tensor("y", [CH, S, D], F32, kind="ExternalOutput").ap()

    consts = ctx.enter_context(tc.tile_pool(name="consts", bufs=1))
    chunkp = ctx.enter_context(tc.tile_pool(name="chunk", bufs=1))
    epool = ctx.enter_context(tc.tile_pool(name="epool", bufs=2))
    ypool = ctx.enter_context(tc.tile_pool(name="ypool", bufs=3))
    small = ctx.enter_context(tc.tile_pool(name="small", bufs=4))
    ps_proj = ctx.enter_context(tc.tile_pool(name="ps_proj", bufs=1, space="PSUM"))
    ps_score = ctx.enter_context(tc.tile_pool(name="ps_score", bufs=3, space="PSUM"))
    ps_attn = ctx.enter_context(tc.tile_pool(name="ps_attn", bufs=1, space="PSUM"))

    wb = {n: consts.tile([128, 2048], BF16, tag=n, name=f"w_{n}")
          for n in ("wbq", "wbk", "wbv")}
    bqt = consts.tile([128, 4], F32, tag="bqt")
    bkt = consts.tile([128, 4], F32, tag="bkt")
    bvb = consts.tile([128, D], F32, tag="bvb")
    gb = consts.tile([128, D], F32, tag="gb")
    bb = consts.tile([128, D], F32, tag="bb")
    lns = consts.tile([128, 1], F32, tag="lns")

    def consts_dma():
        for n in ("wbq", "wbk"):
            nc.sync.dma_start(wb[n][:], wb_d[n][:])
        nc.sync.dma_start(bqt[:], bqt_d[:])
        nc.sync.dma_start(bkt[:], bkt_d[:])
        nc.vector.memset(lns[:], LN_S)

    def consts_dma_late():
        nc.sync.dma_start(wb["wbv"][:], wb_d["wbv"][:])
        nc.sync.dma_start(bvb[:], bvb_d[:])
        nc.sync.dma_start(gb[:], gb_d[:])
        nc.sync.dma_start(bb[:], bb_d[:])

    # greedy engine balancer (estimated cumulative us per engine)
    bal = {"act": 0.0, "dve": 0.0}

    def pick_engine():
        return "act" if bal["act"] <= bal["dve"] else "dve"

    st = [{} for _ in range(CH)]

    def loads(c):
        s = st[c]
        s["xtb"] = chunkp.tile([128, 2048], BF16, tag=f"xtb_{c}", name=f"xtb{c}")
        s["xf"] = chunkp.tile([128, 2048], F32, tag=f"xf_{c}", name=f"xf{c}")
        nc.sync.dma_start(s["xtb"][:], xtb_d[c])
        nc.sync.dma_start(s["xf"][:], xf_d[c])
        s["qT"] = chunkp.tile([128, 2048], BF16, tag=f"qT_{c}", name=f"qT{c}")
        s["qTs"] = chunkp.tile([128, 2048], BF16, tag=f"qTs_{c}", name=f"qTs{c}")
        s["kT"] = chunkp.tile([128, 2048], BF16, tag=f"kT_{c}", name=f"kT{c}")
        s["vp"] = chunkp.tile([128, 4 * 520], E4, tag=f"vp_{c}", name=f"vp{c}")
        s["h"] = chunkp.tile([128, 2048], F32, tag=f"h_{c}", name=f"h{c}")

    def proj(c):
        s = st[c]
        xt = s["xtb"]
        for t in range(4):
            for which in ("q", "k", "v"):
                pp = ps_proj.tile([128, D], F32, tag="proj",
                                  name=f"pp{c}_{which}{t}")
                if which == "v":
                    for mt in range(4):
                        nc.tensor.matmul(
                            pp[:],
                            xt[:, 512 * mt + 128 * t:512 * mt + 128 * t + 128],
                            wb["wbv"][:, 512 * mt:512 * (mt + 1)],
                            start=(mt == 0), stop=(mt == 3))
                    blk = s["vp"][:].rearrange("p (t j c) -> p t j c", j=8, c=65)
                    nc.vector.tensor_tensor(
                        blk[:, t, :, 0:64],
                        pp[:].rearrange("p (j c) -> p j c", c=64),
                        bvb[:].rearrange("p (j c) -> p j c", c=64), op=ALU.add)
                    nc.vector.memset(blk[:, t, :, 64], 1.0)
                else:
                    wname = "wbq" if which == "q" else "wbk"
                    dst = s["qT"] if which == "q" else s["kT"]
                    bias = bqt if which == "q" else bkt
                    for mt in range(4):
                        nc.tensor.matmul(
                            pp[:],
                            wb[wname][:, 512 * mt + 128 * t:512 * mt + 128 * t + 128],
                            xt[:, 512 * mt:512 * (mt + 1)],
                            start=(mt == 0), stop=(mt == 3))
                    eng = pick_engine()
                    if eng == "act":
                        nc.scalar.activation(dst[:, 512 * t:512 * (t + 1)],
                                             pp[:], ACTF.Identity,
                                             bias=bias[:, t:t + 1])
                        bal["act"] += C_ACT_CONV
                    else:
                        nc.vector.tensor_scalar(dst[:, 512 * t:512 * (t + 1)],
                                                pp[:], bias[:, t:t + 1], None,
                                                op0=ALU.add)
                        bal["dve"] += C_DVE_CONV
                    if which == "q":
                        # partition-swapped copy so score matmul rhs can sit at
                        # either PE tile row base (baseline qTs trick)
                        nc.sync.dma_start(
                            s["qTs"][64:128, 512 * t:512 * (t + 1)],
                            dst[0:64, 512 * t:512 * (t + 1)])
                        nc.sync.dma_start(
                            s["qTs"][0:64, 512 * t:512 * (t + 1)],
                            dst[64:128, 512 * t:512 * (t + 1)])

    # DVE's fixed non-exp work, spread as a per-exp-tile handicap so the
    # greedy split tilts toward ACT smoothly instead of in one early burst
    fixed_dve = CH * (C_DVE_LN + 32 * C_DVE_FIN + 4 * C_DVE_VCONV)
    handicap = fixed_dve / (CH * 128.0)

    def emit_exp(dst, ps):
        bal["dve"] += handicap
        eng = pick_engine()
        if eng == "act":
            nc.scalar.activation(dst, ps, ACTF.Exp, scale=0.125, bias=lns[:])
            bal["act"] += C_ACT_EXP
        else:
            nc.vector.tensor_scalar(dst.bitcast(I8), ps, EXP_A, EXP_B,
                                    op0=ALU.mult, op1=ALU.add)
            bal["dve"] += C_DVE_EXP

    def strips(c, jp):
        """Scores + exp for jq pair (2jp, 2jp+1): 16 (r,jku) psum pairs.

        bf16 scores, row-packed: jk-even rows via kT[0:64] at PE tile (0,0),
        jk-odd via kT[64:128] at (64,0); qT/qTs supply the rhs at the
        matching partition base (baseline pattern)."""
        s = st[c]
        qT, qTs, kT = s["qT"], s["qTs"], s["kT"]

        def qrhs(jq, par):
            src = qT if (jq % 2) == par else qTs
            return src[64 * par:64 * par + 64,
                       512 * (jq // 2):512 * (jq // 2) + 512]

        ets = []
        for r in range(4):
            for jku in range(4):
                koff = 512 * jku + 128 * r
                ps0 = ps_score.tile([128, 1024], F32, tag="sps",
                                    name=f"s0_{c}_{jp}_{r}_{jku}")
                ps1 = ps_score.tile([128, 1024], F32, tag="sps",
                                    name=f"s1_{c}_{jp}_{r}_{jku}")
                for pjq in range(2):
                    jq = 2 * jp + pjq
                    nc.tensor.matmul(ps0[:, 512 * pjq:512 * (pjq + 1)],
                                     kT[0:64, koff:koff + 128], qrhs(jq, 0),
                                     start=True, stop=True,
                                     tile_position=(0, 0))
                for pjq in range(2):
                    jq = 2 * jp + pjq
                    nc.tensor.matmul(ps1[:, 512 * pjq:512 * (pjq + 1)],
                                     kT[64:128, koff:koff + 128], qrhs(jq, 1),
                                     start=True, stop=True,
                                     tile_position=(64, 0))
                et = epool.tile([128, 2048], E5, tag=f"e{4 * r + jku}",
                                name=f"e_{c}_{jp}_{r}_{jku}")
                emit_exp(et[:, 0:1024], ps0[:])
                emit_exp(et[:, 1024:2048], ps1[:])
                ets.append(et)
        return ets

    def attn_fin(c, jp, ets, use_sps=False):
        """Flipped attention (E^T stationary) + finalize into h.

        use_sps: draw the accumulators from the (then idle) score psum pool
        for 3-deep pipelining — only safe when no more scores will run."""
        s = st[c]
        vp_v = s["vp"][:].rearrange("p (t j c) -> p t j c", j=8, c=65)
        for sb in range(4):
            for pjq in range(2):
                jq = 2 * jp + pjq
                if use_sps:
                    pa = ps_score.tile([128, 1024], F32, tag="sps",
                                       name=f"pa_{c}_{jp}_{sb}_{pjq}")
                else:
                    pa = ps_attn.tile([128, 512], F32, tag="pa",
                                      name=f"pa_{c}_{jp}_{sb}_{pjq}")
                k = 0
                for r in range(4):
                    for jku in range(4):
                        et = ets[4 * r + jku]
                        lhsT = et[:].rearrange("p (i m) -> p i m", i=2)[
                            :, :, 512 * pjq + 128 * sb:512 * pjq + 128 * sb + 128]
                        rhs = vp_v[:, r, 2 * jku:2 * jku + 2, :]
                        nc.tensor.matmul(pa[:, 0:65], lhsT, rhs,
                                         start=(k == 0), stop=(k == 15),
                                         skip_group_check=True, perf_mode=DR)
                        k += 1
                rcp = small.tile([128, 1], F32, tag="rcp",
                                 name=f"rcp_{c}_{jp}_{sb}_{pjq}")
                nc.vector.reciprocal(rcp[:], pa[:, 64:65])
                off = 512 * sb + 64 * jq
                nc.vector.scalar_tensor_tensor(
                    s["h"][:, off:off + 64], pa[:, 0:64], rcp[:],
                    s["xf"][:, off:off + 64], op0=ALU.mult, op1=ALU.add)

    def layer_norm(c):
        """LayerNorm on DVE; Newton rsqrt batched across the 4 s-blocks."""
        s = st[c]
        h = s["h"]
        I32 = mybir.dt.int32
        mvall = small.tile([128, 8], F32, tag="mvall", name=f"mv{c}")
        for b in range(4):
            st6 = small.tile([128, 6], F32, tag="st6", name=f"st6_{c}_{b}")
            nc.vector.bn_stats(st6[:], h[:, 512 * b:512 * (b + 1)])
            nc.vector.bn_aggr(mvall[:, 2 * b:2 * b + 2], st6[:])
        mean4 = mvall[:].rearrange("p (b two) -> p b two", two=2)[:, :, 0]
        var4 = mvall[:].rearrange("p (b two) -> p b two", two=2)[:, :, 1]
        t4 = small.tile([128, 4], F32, tag="t4", name=f"t4_{c}")
        nc.vector.tensor_scalar_add(t4[:], var4, EPS)
        yi = small.tile([128, 4], I32, tag="yi", name=f"yi{c}")
        nc.vector.tensor_scalar(yi[:], t4[:].bitcast(I32), 1, None,
                                op0=ALU.arith_shift_right)
        nc.vector.tensor_scalar(yi[:], yi[:], 0x5F3759DF, -1,
                                op0=ALU.subtract, op1=ALU.mult)
        rstd = small.tile([128, 4], F32, tag="rstd", name=f"rstd{c}")
        nc.vector.tensor_copy(rstd[:], yi[:].bitcast(F32))
        y2 = small.tile([128, 4], F32, tag="y2", name=f"y2_{c}")
        dd = small.tile([128, 4], F32, tag="dd", name=f"dd{c}")
        for _ in range(3):
            nc.vector.tensor_tensor(y2[:], rstd[:], rstd[:], op=ALU.mult)
            nc.vector.tensor_tensor(y2[:], y2[:], t4[:], op=ALU.mult)
            nc.vector.tensor_scalar(dd[:], y2[:], -0.5, 1.5,
                                    op0=ALU.mult, op1=ALU.add)
            nc.vector.tensor_tensor(rstd[:], rstd[:], dd[:], op=ALU.mult)
        bco = small.tile([128, 4], F32, tag="bco", name=f"bco{c}")
        nc.vector.tensor_tensor(bco[:], mean4, rstd[:], op=ALU.mult)
        nc.vector.tensor_scalar_mul(bco[:], bco[:], -1.0)
        for b in range(4):
            yt = ypool.tile([128, D], F32, tag="yt", name=f"yt{c}_{b}")
            # stage 1 (h*rstd + bco) on ACT to shorten the DVE-only tail
            nc.scalar.activation(yt[:], h[:, 512 * b:512 * (b + 1)],
                                 ACTF.Identity, bias=bco[:, b:b + 1],
                                 scale=rstd[:, b:b + 1])
            nc.vector.tensor_tensor(yt[:], yt[:], gb[:], op=ALU.mult)
            nc.vector.tensor_tensor(yt[:], yt[:], bb[:], op=ALU.add)
            nc.sync.dma_start(y_d[c, 128 * b:128 * (b + 1), :], yt[:])

    # ---- emission schedule: PE order = proj(0), scores(0,0..3) with
    # attn(jp-1) slotted between strips, proj(1), attn(0,3), scores(1,*) ...
    consts_dma()
    loads(0)
    consts_dma_late()
    proj(0)
    loads(1)
    pend = None   # (c, jp, ets) awaiting attn+finalize
    for c in range(CH):
        if c == 1:
            proj(1)
            attn_fin(*pend)
            pend = None
            layer_norm(0)
        for jp in range(4):
            ets = strips(c, jp)
            if pend is not None:
                attn_fin(*pend)
            pend = (c, jp, ets)
    attn_fin(*pend, use_sps=True)
    layer_norm(1)


def build():
    if "nc" in _STATE:
        return _STATE["nc"]
    _imports()
    nc = bacc.Bacc("TRN2", target_bir_lowering=False, debug=False,
                   num_devices=N_CORES)
    with tile.TileContext(nc) as tc:
        with ExitStack() as ctx:
            _emit(nc, tc, ctx)
    nc.compile()
    _STATE["nc"] = nc
    return nc


def host_inputs(Wq, bq, Wk, bk, Wv, bv, gamma, beta):
    """Shared per-core constant inputs (everything except x chunks)."""
    bf = ml_dtypes.bfloat16

    def pack_w(W):
        # [p, (mt, m)]: row 128*mt + p of W at free offset 512*mt + m
        W = np.asarray(W, np.float32)
        return np.ascontiguousarray(
            W.reshape(4, 128, 512).transpose(1, 0, 2).reshape(128, 2048)
        ).astype(bf)

    def bias_t(b):
        return np.ascontiguousarray(
            np.asarray(b, np.float32).reshape(4, 128).T)

    return {
        "wbq": pack_w(Wq), "wbk": pack_w(Wk), "wbv": pack_w(Wv),
        "bqt": bias_t(bq), "bkt": bias_t(bk),
        "bvb": np.broadcast_to(np.asarray(bv, np.float32), (128, D)).copy(),
        "gb": np.broadcast_to(np.asarray(gamma, np.float32), (128, D)).copy(),
        "bb": np.broadcast_to(np.asarray(beta, np.float32), (128, D)).copy(),
    }


def kernel(x, Wq, bq, Wk, bk, Wv, bv, gamma, beta):
    _imports()
    nc = build()
    bf = ml_dtypes.bfloat16
    x = np.asarray(x, np.float32)
    B, Sfull, Dm = x.shape
    chunks = x.reshape(B * 8, S, D)  # chunk c = (b = c//8, head = c%8)
    base = host_inputs(Wq=Wq, bq=bq, Wk=Wk, bk=bk, Wv=Wv, bv=bv,
                       gamma=gamma, beta=beta)
    in_maps = []
    for i in range(N_CORES):
        xc = chunks[2 * i:2 * i + 2]                       # [2, 512, 512]
        xtb = np.ascontiguousarray(
            xc.transpose(0, 2, 1).reshape(CH, 4, 128, S).transpose(0, 2, 1, 3)
            .reshape(CH, 128, 2048)).astype(bf)            # x^T, m-tile-major
        xf = np.ascontiguousarray(
            xc.reshape(CH, 4, 128, D).transpose(0, 2, 1, 3)
            .reshape(CH, 128, 2048))                       # residual, s-block-major
        m = dict(base)
        m["xtb"] = xtb
        m["xf"] = xf
        in_maps.append(m)
    res = bass_utils.run_bass_kernel_spmd(nc, in_maps, core_ids=list(range(N_CORES)))
    out_chunks = np.empty((B * 8, S, D), np.float32)
    for i in range(N_CORES):
        out_chunks[2 * i:2 * i + 2] = res.results[i]["y"]
    return out_chunks.reshape(B, Sfull, Dm)


# revision 17
# speedup vs baseline: 1.6296x; 1.0494x over previous
"""Trainium2 Bass kernel for fused MHA block (nn_MultiHeadAttention_7636451852747).

Reference math (B=2, S=4096, D=512, H=8, hd=64):
    q = (x @ Wq + bq).view(B, H, 4096, 64)   # torch-style view, no transpose
    scores = q @ k^T / 8; attn = softmax(scores) @ v -> reshape(B, S, D)
    y = LayerNorm(x + attn) * gamma + beta

Structure: the .view means head h of batch b reads only rows [512h, 512h+512)
of x[b]; the problem splits into 16 independent [512,512] chunks, 2 per core.

This version is built around the TRN2 cost model's two dominant terms:
  * PE matmul cost = out_free_rows * cycles_per_row; fp8 DoubleRow runs at
    0.5 cycles/row and contracts 2 k-tiles per pass.  All matmuls (proj,
    scores, attn) are fp8e4m3 DoubleRow.  The attention matmul is flipped
    (E^T as the stationary operand) so the output lands in natural [s, dv]
    layout: no PE transposes, no psum->sbuf attn copy, denominator rides as
    a 65th rhs column of ones.
  * Softmax exp of 16.8M scores/chunk is the bottleneck: split across ACT
    (true exp -> fp8e5m2, bias = ln(scale)) and DVE (Schraudolph bit-trick:
    round(score*A + B) as int8 IS the e5m2 weight, scale-matched to ACT).
    Both read the scores psum directly; a greedy cost balancer assigns
    tiles so both engines stay saturated.
GPSIMD cannot touch PSUM on TRN2, so it stays idle; LayerNorm runs on DVE
(bn_stats + Newton rsqrt) as in the baseline.
"""
import numpy as np
import ml_dtypes
from contextlib import ExitStack

_STATE = {}


def _imports():
    global bass, bacc, tile, mybir, bass_utils, F32, BF16, I8, E4, E5, ALU, ACTF, DR
    import concourse.bass as bass
    import concourse.bacc as bacc
    import concourse.tile as tile
    from concourse import mybir
    from concourse import bass_utils
    F32 = mybir.dt.float32
    BF16 = mybir.dt.bfloat16
    I8 = mybir.dt.int8
    E4 = mybir.dt.float8e4
    E5 = mybir.dt.float8e5
    ALU = mybir.AluOpType
    ACTF = mybir.ActivationFunctionType
    DR = mybir.MatmulPerfMode.DoubleRow


N_CORES = 8
CH = 2           # chunks per core
S = 512          # rows per chunk
D = 512          # model dim
EPS = 1e-5

# Schraudolph-e5m2 constants (calibrated offline vs true softmax):
#   i8 = round(score * EXP_A + EXP_B); bits are the e5m2 weight
#   ACT path: exp(score/8 + LN_S) in e5m2 matches the Schraudolph scale.
EXP_A = 4 * np.log2(np.e) / 8        # 0.7213475
EXP_B = 58.0
LN_S = -0.3095

# emit-time engine cost estimates (us) for the greedy ACT/DVE balancer
C_ACT_EXP = 1.098
C_DVE_EXP = 1.262
C_ACT_CONV = 0.672
C_DVE_CONV = 0.730
C_DVE_VCONV = 0.80
C_DVE_FIN = 0.40
C_DVE_LN = 5.2
TAIL_SHIFT = 0.0


def _emit(nc, tc, ctx):
    xtb_d = nc.dram_tensor("xtb", [CH, 128, 2048], BF16, kind="ExternalInput").ap()
    xf_d = nc.dram_tensor("xf", [CH, 128, 2048], F32, kind="ExternalInput").ap()
    wb_d = {n: nc.dram_tensor(n, [128, 2048], BF16, kind="ExternalInput").ap()
            for n in ("wbq", "wbk", "wbv")}
    bqt_d = nc.dram_tensor("bqt", [128, 4], F32, kind="ExternalInput").ap()
    bkt_d = nc.dram_tensor("bkt", [128, 4], F32, kind="ExternalInput").ap()
    bvb_d = nc.dram_tensor("bvb", [128, D], F32, kind="ExternalInput").ap()
    gb_d = nc.dram_tensor("gb", [128, D], F32, kind="ExternalInput").ap()
    bb_d = nc.dram_tensor("bb", [128, D], F32, kind="ExternalInput").ap()
    y_d = nc.dram_tensor("y", [CH, S, D], F32, kind="ExternalOutput").ap()

    consts = ctx.enter_context(tc.tile_pool(name="consts", bufs=1))
    chunkp = ctx.enter_context(tc.tile_pool(name="chunk", bufs=1))
    epool = ctx.enter_context(tc.tile_pool(name="epool", bufs=2))
    ypool = ctx.enter_context(tc.tile_pool(name="ypool", bufs=4))
    small = ctx.enter_context(tc.tile_pool(name="small", bufs=4))
    ps_proj = ctx.enter_context(tc.tile_pool(name="ps_proj", bufs=1, space="PSUM"))
    ps_score = ctx.enter_context(tc.tile_pool(name="ps_score", bufs=3, space="PSUM"))
    ps_attn = ctx.enter_context(tc.tile_pool(name="ps_attn", bufs=1, space="PSUM"))

    wb = {n: consts.tile([128, 2048], BF16, tag=n, name=f"w_{n}")
          for n in ("wbq", "wbk", "wbv")}
    bqt = consts.tile([128, 4], F32, tag="bqt")
    bkt = consts.tile([128, 4], F32, tag="bkt")
    bvb = consts.tile([128, D], F32, tag="bvb")
    gb = consts.tile([128, D], F32, tag="gb")
    bb = consts.tile([128, D], F32, tag="bb")
    lns = consts.tile([128, 1], F32, tag="lns")

    def consts_dma():
        nc.sync.dma_start(wb["wbk"][:], wb_d["wbk"][:])
        nc.vector.memset(lns[:], LN_S)
        # warm the ACT function table while DMAs stream (1.3us one-time load)
        warm = consts.tile([128, 1], F32, tag="warm")
        nc.scalar.activation(warm[:], lns[:], ACTF.Exp)

    def consts_dma_late():
        nc.sync.dma_start(bkt[:], bkt_d[:])
        nc.sync.dma_start(bqt[:], bqt_d[:])
        nc.sync.dma_start(wb["wbq"][:], wb_d["wbq"][:])
        nc.sync.dma_start(wb["wbv"][:], wb_d["wbv"][:])
        nc.sync.dma_start(bvb[:], bvb_d[:])
        nc.sync.dma_start(gb[:], gb_d[:])
        nc.sync.dma_start(bb[:], bb_d[:])

    # greedy engine balancer (estimated cumulative us per engine)
    bal = {"act": 0.0, "dve": 0.0}

    def pick_engine():
        return "act" if bal["act"] <= bal["dve"] else "dve"

    st = [{} for _ in range(CH)]

    def loads(c):
        s = st[c]
        s["xtb"] = chunkp.tile([128, 2048], BF16, tag=f"xtb_{c}", name=f"xtb{c}")
        s["xf"] = chunkp.tile([128, 2048], F32, tag=f"xf_{c}", name=f"xf{c}")
        nc.sync.dma_start(s["xtb"][:], xtb_d[c])
        s["qT"] = chunkp.tile([128, 2048], BF16, tag=f"qT_{c}", name=f"qT{c}")
        s["qTs"] = chunkp.tile([128, 2048], BF16, tag=f"qTs_{c}", name=f"qTs{c}")
        s["kT"] = chunkp.tile([128, 2048], BF16, tag=f"kT_{c}", name=f"kT{c}")
        s["vp"] = chunkp.tile([128, 4 * 520], E4, tag=f"vp_{c}", name=f"vp{c}")
        s["h"] = chunkp.tile([128, 2048], F32, tag=f"h_{c}", name=f"h{c}")

    def proj(c):
        s = st[c]
        xt = s["xtb"]
        n = 0
        for which in ("k", "q", "v"):
            for t in range(4):
                # alternate between the two single-buffer psum pools so the
                # PE->convert chain is double-buffered
                pool_, tag_ = ((ps_proj, "proj") if n % 2 == 0 else
                               (ps_attn, "pa"))
                n += 1
                pp = pool_.tile([128, D], F32, tag=tag_,
                                name=f"pp{c}_{which}{t}")
                if which == "v":
                    for mt in range(4):
                        nc.tensor.matmul(
                            pp[:],
                            xt[:, 512 * mt + 128 * t:512 * mt + 128 * t + 128],
                            wb["wbv"][:, 512 * mt:512 * (mt + 1)],
                            start=(mt == 0), stop=(mt == 3))
                    blk = s["vp"][:].rearrange("p (t j c) -> p t j c", j=8, c=65)
                    nc.vector.tensor_tensor(
                        blk[:, t, :, 0:64],
                        pp[:].rearrange("p (j c) -> p j c", c=64),
                        bvb[:].rearrange("p (j c) -> p j c", c=64), op=ALU.add)
                    nc.vector.memset(blk[:, t, :, 64], 1.0)
                else:
                    wname = "wbq" if which == "q" else "wbk"
                    dst = s["qT"] if which == "q" else s["kT"]
                    bias = bqt if which == "q" else bkt
                    for mt in range(4):
                        nc.tensor.matmul(
                            pp[:],
                            wb[wname][:, 512 * mt + 128 * t:512 * mt + 128 * t + 128],
                            xt[:, 512 * mt:512 * (mt + 1)],
                            start=(mt == 0), stop=(mt == 3))
                    eng = pick_engine()
                    if eng == "act":
                        nc.scalar.activation(dst[:, 512 * t:512 * (t + 1)],
                                             pp[:], ACTF.Identity,
                                             bias=bias[:, t:t + 1])
                        bal["act"] += C_ACT_CONV
                    else:
                        nc.vector.tensor_scalar(dst[:, 512 * t:512 * (t + 1)],
                                                pp[:], bias[:, t:t + 1], None,
                                                op0=ALU.add)
                        bal["dve"] += C_DVE_CONV
                    if which == "q":
                        # partition-swapped copy so score matmul rhs can sit at
                        # either PE tile row base (baseline qTs trick)
                        nc.sync.dma_start(
                            s["qTs"][64:128, 512 * t:512 * (t + 1)],
                            dst[0:64, 512 * t:512 * (t + 1)])
                        nc.sync.dma_start(
                            s["qTs"][0:64, 512 * t:512 * (t + 1)],
                            dst[64:128, 512 * t:512 * (t + 1)])

    # DVE's fixed non-exp work, spread as a per-exp-tile handicap so the
    # greedy split tilts toward ACT smoothly instead of in one early burst
    fixed_dve = CH * (C_DVE_LN + 32 * C_DVE_FIN + 4 * C_DVE_VCONV)
    handicap = fixed_dve / (CH * 128.0)

    def emit_exp(dst, ps):
        bal["dve"] += handicap
        eng = pick_engine()
        if eng == "act":
            nc.scalar.activation(dst, ps, ACTF.Exp, scale=0.125, bias=lns[:])
            bal["act"] += C_ACT_EXP
        else:
            nc.vector.tensor_scalar(dst.bitcast(I8), ps, EXP_A, EXP_B,
                                    op0=ALU.mult, op1=ALU.add)
            bal["dve"] += C_DVE_EXP

    def strips(c, jp):
        """Scores + exp for jq pair (2jp, 2jp+1): 16 (r,jku) psum pairs.

        bf16 scores, row-packed: jk-even rows via kT[0:64] at PE tile (0,0),
        jk-odd via kT[64:128] at (64,0); qT/qTs supply the rhs at the
        matching partition base (baseline pattern)."""
        s = st[c]
        qT, qTs, kT = s["qT"], s["qTs"], s["kT"]

        def qrhs(jq, par):
            src = qT if (jq % 2) == par else qTs
            return src[64 * par:64 * par + 64,
                       512 * (jq // 2):512 * (jq // 2) + 512]

        ets = []
        for r in range(4):
            for jku in range(4):
                koff = 512 * jku + 128 * r
                ps0 = ps_score.tile([128, 1024], F32, tag="sps",
                                    name=f"s0_{c}_{jp}_{r}_{jku}")
                ps1 = ps_score.tile([128, 1024], F32, tag="sps",
                                    name=f"s1_{c}_{jp}_{r}_{jku}")
                for pjq in range(2):
                    jq = 2 * jp + pjq
                    nc.tensor.matmul(ps0[:, 512 * pjq:512 * (pjq + 1)],
                                     kT[0:64, koff:koff + 128], qrhs(jq, 0),
                                     start=True, stop=True,
                                     tile_position=(0, 0))
                for pjq in range(2):
                    jq = 2 * jp + pjq
                    nc.tensor.matmul(ps1[:, 512 * pjq:512 * (pjq + 1)],
                                     kT[64:128, koff:koff + 128], qrhs(jq, 1),
                                     start=True, stop=True,
                                     tile_position=(64, 0))
                et = epool.tile([128, 2048], E5, tag=f"e{4 * r + jku}",
                                name=f"e_{c}_{jp}_{r}_{jku}")
                emit_exp(et[:, 0:1024], ps0[:])
                emit_exp(et[:, 1024:2048], ps1[:])
                ets.append(et)
        return ets

    def attn_fin(c, jp, ets, use_sps=False):
        """Flipped attention (E^T stationary) + finalize into h.

        use_sps: draw the accumulators from the (then idle) score psum pool
        for 3-deep pipelining — only safe when no more scores will run."""
        s = st[c]
        vp_v = s["vp"][:].rearrange("p (t j c) -> p t j c", j=8, c=65)
        for sb in range(4):
            for pjq in range(2):
                jq = 2 * jp + pjq
                if use_sps:
                    pa = ps_score.tile([128, 1024], F32, tag="sps",
                                       name=f"pa_{c}_{jp}_{sb}_{pjq}")
                else:
                    pa = ps_attn.tile([128, 512], F32, tag="pa",
                                      name=f"pa_{c}_{jp}_{sb}_{pjq}")
                k = 0
                for r in range(4):
                    for jku in range(4):
                        et = ets[4 * r + jku]
                        lhsT = et[:].rearrange("p (i m) -> p i m", i=2)[
                            :, :, 512 * pjq + 128 * sb:512 * pjq + 128 * sb + 128]
                        rhs = vp_v[:, r, 2 * jku:2 * jku + 2, :]
                        nc.tensor.matmul(pa[:, 0:65], lhsT, rhs,
                                         start=(k == 0), stop=(k == 15),
                                         skip_group_check=True, perf_mode=DR)
                        k += 1
                off = 512 * sb + 64 * jq
                nc.vector.scalar_tensor_tensor(
                    s["h"][:, off:off + 64], pa[:, 0:64], pa[:, 64:65],
                    s["xf"][:, off:off + 64], op0=ALU.divide, op1=ALU.add)

    def layer_norm(c):
        """LayerNorm on DVE; Newton rsqrt batched across the 4 s-blocks."""
        s = st[c]
        h = s["h"]
        I32 = mybir.dt.int32
        mvall = small.tile([128, 8], F32, tag="mvall", name=f"mv{c}")
        for b in range(4):
            st6 = small.tile([128, 6], F32, tag="st6", name=f"st6_{c}_{b}")
            nc.vector.bn_stats(st6[:], h[:, 512 * b:512 * (b + 1)])
            nc.vector.bn_aggr(mvall[:, 2 * b:2 * b + 2], st6[:])
        mean4 = mvall[:].rearrange("p (b two) -> p b two", two=2)[:, :, 0]
        var4 = mvall[:].rearrange("p (b two) -> p b two", two=2)[:, :, 1]
        t4 = small.tile([128, 4], F32, tag="t4", name=f"t4_{c}")
        nc.vector.tensor_scalar_add(t4[:], var4, EPS)
        sq4 = small.tile([128, 4], F32, tag="sq4", name=f"sq4_{c}")
        nc.scalar.activation(sq4[:], t4[:], ACTF.Sqrt)
        rstd = small.tile([128, 4], F32, tag="rstd", name=f"rstd{c}")
        nc.vector.reciprocal(rstd[:], sq4[:])
        bco = small.tile([128, 4], F32, tag="bco", name=f"bco{c}")
        nc.vector.tensor_tensor(bco[:], mean4, rstd[:], op=ALU.mult)
        nc.vector.tensor_scalar_mul(bco[:], bco[:], -1.0)
        for b in range(4):
            yt = ypool.tile([128, D], F32, tag="yt", name=f"yt{c}_{b}")
            # stage 1 (h*rstd + bco) on ACT to shorten the DVE-only tail
            nc.scalar.activation(yt[:], h[:, 512 * b:512 * (b + 1)],
                                 ACTF.Identity, bias=bco[:, b:b + 1],
                                 scale=rstd[:, b:b + 1])
            nc.vector.tensor_tensor(yt[:], yt[:], gb[:], op=ALU.mult)
            nc.gpsimd.tensor_tensor(yt[:], yt[:], bb[:], op=ALU.add)
            nc.sync.dma_start(y_d[c, 128 * b:128 * (b + 1), :], yt[:])

    # ---- emission schedule: PE order = proj(0), scores(0,0..3) with
    # attn(jp-1) slotted between strips, proj(1), attn(0,3), scores(1,*) ...
    # The final strips gets an extra DVE handicap so ACT absorbs more of the
    # last exp wave while DVE runs finalize+LN with nothing after it.
    consts_dma()
    loads(0)
    consts_dma_late()
    proj(0)
    loads(1)
    nc.sync.dma_start(st[0]["xf"][:], xf_d[0])
    nc.sync.dma_start(st[1]["xf"][:], xf_d[1])
    pend = None   # (c, jp, ets) awaiting attn+finalize
    for c in range(CH):
        if c == 1:
            proj(1)
            attn_fin(*pend)
            pend = None
            layer_norm(0)
        for jp in range(4):
            if c == CH - 1 and jp == 3:
                bal["dve"] += TAIL_SHIFT
            ets = strips(c, jp)
            if pend is not None:
                attn_fin(*pend)
            pend = (c, jp, ets)
    attn_fin(*pend, use_sps=True)
    layer_norm(1)


def build():
    if "nc" in _STATE:
        return _STATE["nc"]
    _imports()
    nc = bacc.Bacc("TRN2", target_bir_lowering=False, debug=False,
                   num_devices=N_CORES)
    with tile.TileContext(nc) as tc:
        with ExitStack() as ctx:
            _emit(nc, tc, ctx)
    nc.compile()
    _STATE["nc"] = nc
    return nc


def host_inputs(Wq, bq, Wk, bk, Wv, bv, gamma, beta):
    """Shared per-core constant inputs (everything except x chunks)."""
    bf = ml_dtypes.bfloat16

    def pack_w(W):
        # [p, (mt, m)]: row 128*mt + p of W at free offset 512*mt + m
        W = np.asarray(W, np.float32)
        return np.ascontiguousarray(
            W.reshape(4, 128, 512).transpose(1, 0, 2).reshape(128, 2048)
        ).astype(bf)

    def bias_t(b):
        return np.ascontiguousarray(
            np.asarray(b, np.float32).reshape(4, 128).T)

    return {
        "wbq": pack_w(Wq), "wbk": pack_w(Wk), "wbv": pack_w(Wv),
        "bqt": bias_t(bq), "bkt": bias_t(bk),
        "bvb": np.broadcast_to(np.asarray(bv, np.float32), (128, D)).copy(),
        "gb": np.broadcast_to(np.asarray(gamma, np.float32), (128, D)).copy(),
        "bb": np.broadcast_to(np.asarray(beta, np.float32), (128, D)).copy(),
    }


def kernel(x, Wq, bq, Wk, bk, Wv, bv, gamma, beta):
    _imports()
    nc = build()
    bf = ml_dtypes.bfloat16
    x = np.asarray(x, np.float32)
    B, Sfull, Dm = x.shape
    chunks = x.reshape(B * 8, S, D)  # chunk c = (b = c//8, head = c%8)
    base = host_inputs(Wq=Wq, bq=bq, Wk=Wk, bk=bk, Wv=Wv, bv=bv,
                       gamma=gamma, beta=beta)
    in_maps = []
    for i in range(N_CORES):
        xc = chunks[2 * i:2 * i + 2]                       # [2, 512, 512]
        xtb = np.ascontiguousarray(
            xc.transpose(0, 2, 1).reshape(CH, 4, 128, S).transpose(0, 2, 1, 3)
            .reshape(CH, 128, 2048)).astype(bf)            # x^T, m-tile-major
        xf = np.ascontiguousarray(
            xc.reshape(CH, 4, 128, D).transpose(0, 2, 1, 3)
            .reshape(CH, 128, 2048))                       # residual, s-block-major
        m = dict(base)
        m["xtb"] = xtb
        m["xf"] = xf
        in_maps.append(m)
    res = bass_utils.run_bass_kernel_spmd(nc, in_maps, core_ids=list(range(N_CORES)))
    out_chunks = np.empty((B * 8, S, D), np.float32)
    for i in range(N_CORES):
        out_chunks[2 * i:2 * i + 2] = res.results[i]["y"]
    return out_chunks.reshape(B, Sfull, Dm)
